# revision 1
# baseline (speedup 1.0000x reference)
"""DeltaHebbianBlock Trainium2 kernel.

Sharding: 8 cores = (B=2) x (H=4) head-parallel. Each core computes its
head's delta-rule chunked scan and the partial output projection
partial_bh = (alpha_h * o_bh) @ Wr_h^T  (8192 x 1024).
Host gathers: out[b] = x[b] + sum_h partial[b,h].

Per-core pipeline (T=8192, d=256, C=64, 128 chunks, 8 quarter-passes):
  P1: DMA-transpose x -> xT (bf16), v = x @ WwT (bf16 mm, f32 psum),
      rk = normalize(x_h), rkT via PE transpose, wk = shift(rk) via SBUF DMA.
  P2: per chunk-pair (block-diag 128x128): grams W = wk wk^T, intraT;
      A^T = (I+C0)(I+C1)(I+C2) truncated nilpotent chain (exact to A0^7);
      v_corr = A v, wk_corrT = (A wk)^T; rkgT, wkgN scalings.
  P3: sequential scan: v_new = v_corr - wk_corr S; o = rkg S + intra v_new;
      S = gC S + wkgN^T v_new.
  P4: oT via PE transpose; partial = oT^T @ (alpha WrT) (bf16 mm).
"""
import os
import numpy as np
import ml_dtypes
from contextlib import ExitStack

import concourse.bass as bass
import concourse.mybir as mybir
import concourse.tile as tile
from concourse import bacc, bass_utils

B, T, D = 2, 8192, 1024
H, d, C = 4, 256, 64
NCH = T // C          # 128 chunks
NQ = 8                # quarter passes
QT = T // NQ          # 1024 tokens per pass
QTT = QT // 128       # 8 p-tiles per pass
QCH = QT // C         # 16 chunks per pass
QPR = QCH // 2        # 8 pairs per pass

F32 = mybir.dt.float32
BF16 = mybir.dt.bfloat16


def _build():
    nc = bacc.Bacc("TRN2", target_bir_lowering=False, debug=False, num_devices=int(os.environ.get("K_NCORES", "8")))
    xbf = nc.dram_tensor("xbf", (T, D), BF16, kind="ExternalInput")
    wwt = nc.dram_tensor("wwt", (D, d), BF16, kind="ExternalInput")
    wrt = nc.dram_tensor("wrt", (d, D), BF16, kind="ExternalInput")
    mb_d = nc.dram_tensor("mb", (128, 128), F32, kind="ExternalInput")
    mc_d = nc.dram_tensor("mc", (128, 128), F32, kind="ExternalInput")
    mit_d = nc.dram_tensor("mit", (128, 128), F32, kind="ExternalInput")
    id_d = nc.dram_tensor("ident", (128, 128), BF16, kind="ExternalInput")
    gpb_d = nc.dram_tensor("gpbf", (128, QT), BF16, kind="ExternalInput")
    gpt_d = nc.dram_tensor("gpt", (128, 1), F32, kind="ExternalInput")
    gcv_d = nc.dram_tensor("gcv", (128, 1), F32, kind="ExternalInput")
    part_d = nc.dram_tensor("partial", (T, D), F32, kind="ExternalOutput")

    with ExitStack() as ctx:
        tc = ctx.enter_context(tile.TileContext(nc))
        consts = ctx.enter_context(tc.tile_pool(name="consts", bufs=1))
        big = ctx.enter_context(tc.tile_pool(name="big", bufs=1))
        qbuf = ctx.enter_context(tc.tile_pool(name="qbuf", bufs=1))
        qbuf2 = ctx.enter_context(tc.tile_pool(name="qbuf2", bufs=2))
        chain = ctx.enter_context(tc.tile_pool(name="chain", bufs=2))
        vnewp = ctx.enter_context(tc.tile_pool(name="vnewp", bufs=4))
        stage = ctx.enter_context(tc.tile_pool(name="stage", bufs=2))
        scr = ctx.enter_context(tc.tile_pool(name="scr", bufs=2))
        ps_g = ctx.enter_context(tc.tile_pool(name="ps_g", bufs=2, space="PSUM"))
        ps_a = ctx.enter_context(tc.tile_pool(name="ps_a", bufs=3, space="PSUM"))
        ps_s = ctx.enter_context(tc.tile_pool(name="ps_s", bufs=1, space="PSUM"))
        ps_p = ctx.enter_context(tc.tile_pool(name="ps_p", bufs=2, space="PSUM"))

        # ---- constants / weights in SBUF ----
        wwt_s = consts.tile([128, 8, d], BF16)
        nc.sync.dma_start(wwt_s[:], wwt.ap().rearrange("(kb p) j -> p kb j", p=128))
        wrt_s = consts.tile([128, 2, D], BF16)
        nc.sync.dma_start(wrt_s[:], wrt.ap().rearrange("(kt p) n -> p kt n", p=128))
        mb_s = consts.tile([128, 128], F32)
        nc.sync.dma_start(mb_s[:], mb_d.ap())
        mc_s = consts.tile([128, 128], F32)
        nc.sync.dma_start(mc_s[:], mc_d.ap())
        mit_s = consts.tile([128, 128], F32)
        nc.sync.dma_start(mit_s[:], mit_d.ap())
        id_s = consts.tile([128, 128], BF16)
        nc.sync.dma_start(id_s[:], id_d.ap())
        gpb_s = consts.tile([128, QT], BF16)
        nc.sync.dma_start(gpb_s[:], gpb_d.ap())
        gpt_s = consts.tile([128, 1], F32)
        nc.sync.dma_start(gpt_s[:], gpt_d.ap())
        gcv_s = consts.tile([128, 1], F32)
        nc.sync.dma_start(gcv_s[:], gcv_d.ap())

        # ---- full-T persistent (bf16) ----
        rk = big.tile([128, T // 128, d], BF16)       # 4MB
        wk = big.tile([128, T // 128, d], BF16)       # 4MB
        rkT = big.tile([128, 2, T + 1], BF16)         # 4MB (col 0 = zero pad)
        S_bf = big.tile([128, 2, d], BF16)
        nc.gpsimd.memset(S_bf[:], 0.0)
        nc.gpsimd.memset(rkT[:, :, 0:1], 0.0)
        nc.gpsimd.memset(wk[0:1, 0:1, :], 0.0)

        for q in range(NQ):
            if os.environ.get("K_STOP") == "consts":
                break
            qt0 = q * QT          # token offset
            tt0 = q * QTT         # p-tile offset
            # ---------------- P1 ----------------
            xT = qbuf2.tile([128, 8, QT], BF16, tag="xT")
            for kb in range(8):
                nc.sync.dma_start(
                    xT[:, kb, :],
                    xbf.ap()[qt0:qt0 + QT, kb * 128:(kb + 1) * 128],
                    transpose=True)
            if os.environ.get("K_STOP") == "xt":
                continue
            xh = qbuf.tile([128, QTT, d], BF16, tag="xh")
            h_ap = xbf.ap()[qt0:qt0 + QT, :]  # head slice set on host via col offset 0
            nc.sync.dma_start(
                xh[:], h_ap[:, 0:d].rearrange("(tt p) j -> p tt j", p=128))
            if os.environ.get("K_STOP") == "xh":
                continue
            v_nat = qbuf2.tile([128, QTT, d], BF16, tag="v_nat")
            for tt in range(QTT):
                vps = ps_p.tile([128, d], F32, tag="p")
                nkb = int(os.environ.get("K_KB", "8"))
                for kb in range(nkb):
                    nc.tensor.matmul(vps[:], xT[:, kb, tt * 128:(tt + 1) * 128],
                                     wwt_s[:, kb, :], start=(kb == 0), stop=(kb == nkb - 1))
                nc.vector.tensor_copy(v_nat[:, tt, :], vps[:])
            if os.environ.get("K_STOP") == "v":
                continue
            # rk = normalize(xh)
            rklvl = os.environ.get("K_RK", "all")
            for tt in range(QTT):
                sq = scr.tile([128, d], F32, tag="sq")
                ss = scr.tile([128, 1], F32, tag="ss")
                nc.scalar.activation(sq[:], xh[:, tt, :],
                                     mybir.ActivationFunctionType.Square,
                                     accum_out=ss[:])
                if rklvl == "red":
                    continue
                nrm = scr.tile([128, 1], F32, tag="nrm")
                nc.scalar.activation(nrm[:], ss[:], mybir.ActivationFunctionType.Sqrt)
                inv = scr.tile([128, 1], F32, tag="inv")
                nc.vector.reciprocal(inv[:], nrm[:])
                if rklvl == "sqrt":
                    continue
                nc.scalar.activation(rk[:, tt0 + tt, :], xh[:, tt, :],
                                     mybir.ActivationFunctionType.Copy, scale=inv[:])
                if rklvl == "scale":
                    continue
                for kt in range(2):
                    tps = ps_g.tile([128, 128], BF16, tag="g")
                    nc.tensor.transpose(tps[:], rk[:, tt0 + tt, kt * 128:(kt + 1) * 128],
                                        id_s[:])
                    nc.vector.tensor_copy(
                        rkT[:, kt, 1 + qt0 + tt * 128: 1 + qt0 + (tt + 1) * 128], tps[:])
            if os.environ.get("K_STOP") == "rk":
                continue
            # wk = shift(rk) by one row
            nc.sync.dma_start(wk[1:128, tt0:tt0 + QTT, :], rk[0:127, tt0:tt0 + QTT, :])
            lo = max(tt0, 1)
            nc.sync.dma_start(wk[0:1, lo:tt0 + QTT, :], rk[127:128, lo - 1:tt0 + QTT - 1, :])
            # wkgN = wk * gp_tail (per-partition), rkgT = rkT * gp (per-col)
            wkgN = qbuf.tile([128, QTT, d], BF16, tag="wkgN")
            nc.scalar.activation(wkgN[:], wk[:, tt0:tt0 + QTT, :],
                                 mybir.ActivationFunctionType.Copy, scale=gpt_s[:])
            rkgT = qbuf.tile([128, 2, QT], BF16, tag="rkgT")
            for kt in range(2):
                nc.vector.tensor_mul(rkgT[:, kt, :], rkT[:, kt, 1 + qt0:1 + qt0 + QT],
                                     gpb_s[:])
            if os.environ.get("K_STOP") == "p1":
                continue
            # ---------------- P2 ----------------
            AT = qbuf2.tile([128, QPR * 128], BF16, tag="AT")
            inT = qbuf.tile([128, QPR * 128], BF16, tag="inT")
            v_corr = qbuf.tile([128, QTT, d], BF16, tag="v_corr")
            wkcT = qbuf2.tile([128, 2, QT], BF16, tag="wkcT")
            for p in range(QPR):
                w0 = qt0 + p * 128  # global token col of pair window
                gps = ps_g.tile([128, 128], F32, tag="g")
                for kt in range(2):
                    nc.tensor.matmul(gps[:], rkT[:, kt, w0:w0 + 128],
                                     rkT[:, kt, w0:w0 + 128],
                                     start=(kt == 0), stop=(kt == 1))
                B0 = chain.tile([128, 128], BF16, tag="B0")
                nc.vector.tensor_mul(B0[:], gps[:], mb_s[:])
                C0 = chain.tile([128, 128], BF16, tag="C0")
                nc.vector.tensor_mul(C0[:], gps[:], mc_s[:])
                ips = ps_g.tile([128, 128], F32, tag="g")
                for kt in range(2):
                    nc.tensor.matmul(ips[:], rkT[:, kt, w0:w0 + 128],
                                     rkT[:, kt, w0 + 1:w0 + 129],
                                     start=(kt == 0), stop=(kt == 1))
                nc.vector.tensor_mul(inT[:, p * 128:(p + 1) * 128], ips[:], mit_s[:])
                # chain: C1 = C0^2, B1 = C1^T-path, C2 = C1^2
                c1p = ps_g.tile([128, 128], F32, tag="g")
                nc.tensor.matmul(c1p[:], B0[:], C0[:])
                C1 = chain.tile([128, 128], BF16, tag="C1")
                nc.vector.tensor_copy(C1[:], c1p[:])
                b1p = ps_g.tile([128, 128], F32, tag="g")
                nc.tensor.matmul(b1p[:], C0[:], B0[:])
                B1 = chain.tile([128, 128], BF16, tag="B1")
                nc.vector.tensor_copy(B1[:], b1p[:])
                c2p = ps_g.tile([128, 128], F32, tag="g")
                nc.tensor.matmul(c2p[:], B1[:], C1[:])
                C2 = chain.tile([128, 128], BF16, tag="C2")
                nc.vector.tensor_copy(C2[:], c2p[:])
                G0 = chain.tile([128, 128], BF16, tag="G0")
                nc.vector.tensor_add(G0[:], B0[:], id_s[:])
                Gh0 = chain.tile([128, 128], BF16, tag="Gh0")
                nc.vector.tensor_add(Gh0[:], C0[:], id_s[:])
                g1p = ps_g.tile([128, 128], F32, tag="g")
                nc.tensor.matmul(g1p[:], G0[:], C1[:])
                G1h = chain.tile([128, 128], BF16, tag="G1h")
                nc.vector.tensor_add(G1h[:], g1p[:], Gh0[:])
                g1tp = ps_g.tile([128, 128], BF16, tag="g")
                nc.tensor.transpose(g1tp[:], G1h[:], id_s[:])
                G1 = chain.tile([128, 128], BF16, tag="G1")
                nc.vector.tensor_copy(G1[:], g1tp[:])
                g2p = ps_g.tile([128, 128], F32, tag="g")
                nc.tensor.matmul(g2p[:], G1[:], C2[:])
                nc.vector.tensor_add(AT[:, p * 128:(p + 1) * 128], g2p[:], G1h[:])
                # applications
                vcp = ps_a.tile([128, d], F32, tag="a")
                nc.tensor.matmul(vcp[:], AT[:, p * 128:(p + 1) * 128], v_nat[:, p, :])
                nc.vector.tensor_copy(v_corr[:, p, :], vcp[:])
                for jb in range(2):
                    wcp = ps_a.tile([128, 128], F32, tag="a")
                    nc.tensor.matmul(wcp[:], wk[:, tt0 + p, jb * 128:(jb + 1) * 128],
                                     AT[:, p * 128:(p + 1) * 128])
                    nc.vector.tensor_copy(wkcT[:, jb, p * 128:(p + 1) * 128], wcp[:])
            if os.environ.get("K_STOP") == "p2":
                continue
            # ---------------- P3: scan ----------------
            o_nat = qbuf2.tile([128, QTT, d], BF16, tag="o_nat")
            for cq in range(QCH):
                tt = cq // 2
                poff = (cq % 2) * 64
                gcol = cq * 64
                p = cq // 2
                sl = slice(poff, poff + 64)
                vnp = ps_a.tile([128, d], F32, tag="a")
                for jb in range(2):
                    nc.tensor.matmul(vnp[sl, :], wkcT[:, jb, gcol:gcol + 64], S_bf[:, jb, :],
                                     start=(jb == 0), stop=(jb == 1))
                vnew = vnewp.tile([128, d], BF16, tag="vnew")
                nc.vector.scalar_tensor_tensor(
                    vnew[sl, :], vnp[sl, :], -1.0, v_corr[sl, tt, :],
                    mybir.AluOpType.mult, mybir.AluOpType.add)
                ops = ps_a.tile([128, d], F32, tag="a")
                for jb in range(2):
                    nc.tensor.matmul(ops[sl, :], rkgT[:, jb, gcol:gcol + 64], S_bf[:, jb, :],
                                     start=(jb == 0), stop=False)
                nc.tensor.matmul(ops[sl, :], inT[sl, p * 128 + poff:p * 128 + poff + 64],
                                 vnew[sl, :], start=False, stop=True)
                nc.scalar.activation(o_nat[sl, tt, :], ops[sl, :],
                                     mybir.ActivationFunctionType.Copy)
                sup = ps_s.tile([128, 2 * d], F32, tag="s")
                for jb in range(2):
                    nc.tensor.matmul(sup[:, jb * d:(jb + 1) * d],
                                     wkgN[sl, tt, jb * 128:(jb + 1) * 128],
                                     vnew[sl, :])
                nc.vector.scalar_tensor_tensor(
                    S_bf[:, :, :], S_bf[:, :, :], gcv_s[:],
                    sup[:].rearrange("p (jb n) -> p jb n", jb=2),
                    mybir.AluOpType.mult, mybir.AluOpType.add)
            if os.environ.get("K_STOP") == "p3":
                continue
            # ---------------- P4 ----------------
            oT = qbuf.tile([128, 2, QT], BF16, tag="oT")
            for p in range(QPR):
                for kt in range(2):
                    otp = ps_g.tile([128, 128], BF16, tag="g")
                    nc.tensor.transpose(otp[:], o_nat[:, p, kt * 128:(kt + 1) * 128], id_s[:])
                    nc.vector.tensor_copy(oT[:, kt, p * 128:(p + 1) * 128], otp[:])
                st = stage.tile([128, D], F32, tag="st")
                for nh in range(2):
                    pps = ps_p.tile([128, 512], F32, tag="p")
                    for kt in range(2):
                        nc.tensor.matmul(pps[:], oT[:, kt, p * 128:(p + 1) * 128],
                                         wrt_s[:, kt, nh * 512:(nh + 1) * 512],
                                         start=(kt == 0), stop=(kt == 1))
                    nc.vector.tensor_copy(st[:, nh * 512:(nh + 1) * 512], pps[:])
                nc.sync.dma_start(
                    part_d.ap()[qt0 + p * 128: qt0 + (p + 1) * 128, :], st[:])
    nc.compile()
    return nc


_NC = None
LAST_EXEC_NS = None
LAST_TRACE = None


def _bf16(a):
    return np.ascontiguousarray(a.astype(ml_dtypes.bfloat16))


def kernel(out, Ww, Wr, decay, log_alpha):
    global _NC
    out = np.asarray(out, dtype=np.float32)
    Ww = np.asarray(Ww, dtype=np.float32)
    Wr = np.asarray(Wr, dtype=np.float32)
    decay = np.asarray(decay, dtype=np.float32)
    log_alpha = np.asarray(log_alpha, dtype=np.float32)
    gamma = 1.0 / (1.0 + np.exp(-decay.astype(np.float64)))
    alpha = np.exp(log_alpha.astype(np.float64))

    if _NC is None:
        _NC = _build()
    nc = _NC

    p64 = np.arange(64)
    in_maps = []
    for c in range(8):
        b, h = c // 4, c % 4
        g = gamma[h]
        # x with head-h channels rotated to the front so the kernel's
        # xh slice [:, 0:d] is the head slice (v-proj uses matching
        # rotated WwT so the product is unchanged).
        xr = np.roll(out[b], -h * d, axis=1)
        wwr = np.roll(Ww[h * d:(h + 1) * d, :], -h * d, axis=1).T  # (D, d)
        wrs = (alpha[h] * Wr[:, h * d:(h + 1) * d]).T              # (d, D)
        Ls = np.tril(g ** np.maximum(p64[:, None] - p64[None, :], 0), -1)
        mbB = (-Ls).astype(np.float32)
        mitB = np.triu(g ** np.maximum(p64[None, :] - p64[:, None], 0), 1).astype(np.float32)
        z = np.zeros((64, 64), np.float32)
        mb = np.block([[mbB, z], [z, mbB]])
        mit = np.block([[mitB, z], [z, mitB]])
        gp = (g ** p64).astype(np.float32)
        gpb = np.tile(gp, QT // 64)[None, :].repeat(128, 0)
        gpt = (g ** (63 - (np.arange(128) % 64)))[:, None].astype(np.float32)
        gcv = np.full((128, 1), g ** 64, np.float32)
        in_maps.append({
            "xbf": _bf16(xr),
            "wwt": _bf16(wwr),
            "wrt": _bf16(wrs),
            "mb": mb, "mc": np.ascontiguousarray(mb.T),
            "mit": mit,
            "ident": _bf16(np.eye(128, dtype=np.float32)),
            "gpbf": _bf16(gpb),
            "gpt": gpt, "gcv": gcv,
        })

    ncore = int(os.environ.get("K_NCORES", "8"))
    res = bass_utils.run_bass_kernel_spmd(
        nc, in_maps[:ncore], core_ids=list(range(ncore)),
        trace=bool(os.environ.get("K_TRACE")))
    global LAST_EXEC_NS, LAST_TRACE
    LAST_EXEC_NS = res.exec_time_ns
    LAST_TRACE = res.instructions_and_trace
    final = out.copy()
    for c in range(len(res.results)):
        b = c // 4
        final[b] += res.results[c]["partial"]
    return final



# revision 20
# speedup vs baseline: 2.4659x; 2.4659x over previous
"""DeltaHebbianBlock Trainium2 kernel (v3).

Sharding: 8 cores = (B=2) x (H=4). Each core runs its head's delta-rule
chunked scan (C=128, degree-3 UT chain) and the partial output projection
partial_bh = (alpha_h * o_bh) @ Wr_h^T. Host: out[b] = x[b] + sum_h partial.

Fully fused slot schedule per quarter (QT=1024, 8 chunks of C=128):
slot n runs P2(q,n) chain (pair-batched drains), P3(q,n-2) scan,
P4(q-1,n) out-proj, P1(q+1,n) v-proj/rk.  Tricks: wkcT stored negated so
vnew accumulates fully in PSUM (no TSP); S decay folded into the sup
matmul group via gcv*I; scan emits oT directly (no o transposes).
"""
import os
import numpy as np
import ml_dtypes
from contextlib import ExitStack

import concourse.bass as bass
import concourse.mybir as mybir
import concourse.tile as tile
from concourse import bacc, bass_utils

B, T, D = 2, 8192, 1024
H, d, C = 4, 256, 128
NQ = 8                # quarter passes
QT = T // NQ          # 1024 tokens per pass
NCH = QT // C         # 8 chunks per pass

F32 = mybir.dt.float32
BF16 = mybir.dt.bfloat16
MULT = mybir.AluOpType.mult
ADD = mybir.AluOpType.add
ACT_COPY = None  # set in _build


def _build():
    nc = bacc.Bacc("TRN2", target_bir_lowering=False, debug=False,
                   num_devices=int(os.environ.get("K_NCORES", "8")))
    xT_d = nc.dram_tensor("xt", (D, T), BF16, kind="ExternalInput")
    xh_d = nc.dram_tensor("xh", (T, d), BF16, kind="ExternalInput")
    wwt_d = nc.dram_tensor("wwt", (D, d), BF16, kind="ExternalInput")
    wrt_d = nc.dram_tensor("wrt", (d, D), BF16, kind="ExternalInput")
    mb_d = nc.dram_tensor("mb", (C, C), F32, kind="ExternalInput")
    mc_d = nc.dram_tensor("mc", (C, C), F32, kind="ExternalInput")
    mit_d = nc.dram_tensor("mit", (C, C), F32, kind="ExternalInput")
    id_d = nc.dram_tensor("ident", (128, 128), BF16, kind="ExternalInput")
    gcvid_d = nc.dram_tensor("gcvid", (128, 128), BF16, kind="ExternalInput")
    gpb_d = nc.dram_tensor("gpbf", (128, QT), BF16, kind="ExternalInput")
    gpt_d = nc.dram_tensor("gpt", (128, 1), F32, kind="ExternalInput")
    part_d = nc.dram_tensor("partial", (T, D), BF16, kind="ExternalOutput")

    COPY = mybir.ActivationFunctionType.Copy
    SQRT = mybir.ActivationFunctionType.Sqrt
    GP = nc.gpsimd if os.environ.get("K_POOL", "1") == "1" else nc.vector

    with ExitStack() as ctx:
        tc = ctx.enter_context(tile.TileContext(nc))
        consts = ctx.enter_context(tc.tile_pool(name="consts", bufs=1))
        qx = ctx.enter_context(tc.tile_pool(name="qx", bufs=2))
        qa = ctx.enter_context(tc.tile_pool(name="qa", bufs=2))
        chp = ctx.enter_context(tc.tile_pool(name="chp", bufs=2))
        vnw = ctx.enter_context(tc.tile_pool(name="vnw", bufs=3))
        st_p = ctx.enter_context(tc.tile_pool(name="stp", bufs=2))
        scr = ctx.enter_context(tc.tile_pool(name="scr", bufs=2))
        ps = ctx.enter_context(tc.tile_pool(name="ps", bufs=1, space="PSUM"))

        # ---- constants / weights in SBUF ----
        wwt_s = consts.tile([128, 8, d], BF16)
        nc.sync.dma_start(wwt_s[:], wwt_d.ap().rearrange("(kb p) j -> p kb j", p=128))
        wrt_s = consts.tile([128, 2, D], BF16)
        nc.sync.dma_start(wrt_s[:], wrt_d.ap().rearrange("(kt p) n -> p kt n", p=128))
        mb2_s = consts.tile([128, 2, 128], F32)
        mc2_s = consts.tile([128, 2, 128], F32)
        mit2_s = consts.tile([128, 2, 128], F32)
        id2_s = consts.tile([128, 2, 128], BF16)
        for ch in range(2):
            nc.sync.dma_start(mb2_s[:, ch, :], mb_d.ap())
            nc.sync.dma_start(mc2_s[:, ch, :], mc_d.ap())
            nc.sync.dma_start(mit2_s[:, ch, :], mit_d.ap())
            nc.sync.dma_start(id2_s[:, ch, :], id_d.ap())
        id_s = consts.tile([128, 128], BF16)
        nc.sync.dma_start(id_s[:], id_d.ap())
        gcvid_s = consts.tile([128, 128], BF16)
        nc.sync.dma_start(gcvid_s[:], gcvid_d.ap())
        gpb_s = consts.tile([128, QT], BF16)
        nc.sync.dma_start(gpb_s[:], gpb_d.ap())
        gpt_s = consts.tile([128, 1], F32)
        nc.sync.dma_start(gpt_s[:], gpt_d.ap())

        S_bf = consts.tile([128, 2, d], BF16)
        nc.gpsimd.memset(S_bf[:], 0.0)

        QS = {}  # per-quarter tile sets

        def qtiles(qq):
            if qq in QS:
                return QS[qq]
            t = {}
            t["xT"] = qx.tile([128, 8, QT], BF16, tag="xT", name="xT")
            t["xh"] = qa.tile([128, 8, d], BF16, tag="xh", name="xh")
            t["v_nat"] = qa.tile([128, 8, d], BF16, tag="v_nat", name="v_nat")
            t["nrm2"] = scr.tile([128, 8], F32, tag="nrm2", name="nrm2")
            t["nrm"] = scr.tile([128, 8], F32, tag="nrm", name="nrm")
            t["inv"] = scr.tile([128, 8], F32, tag="inv", name="inv")
            t["rk"] = qa.tile([128, 8, d], BF16, tag="rk", name="rk")
            t["wk"] = qa.tile([128, 8, d], BF16, tag="wk", name="wk")
            t["wkgN"] = qa.tile([128, 8, d], BF16, tag="wkgN", name="wkgN")
            t["rkT"] = qa.tile([128, 2, QT + 1], BF16, tag="rkT", name="rkT")
            t["rkgT"] = qa.tile([128, 2, QT], BF16, tag="rkgT", name="rkgT")
            t["wkcT"] = qa.tile([128, 2, QT], BF16, tag="wkcT", name="wkcT")
            t["inT"] = qa.tile([128, NCH, C], BF16, tag="inT", name="inT")
            t["AT"] = qa.tile([128, NCH, C], BF16, tag="AT", name="AT")
            t["oT"] = qa.tile([128, 2, QT], BF16, tag="oT", name="oT")
            QS[qq] = t
            return t

        def loads(qq):
            t = qtiles(qq)
            qt0 = qq * QT
            nc.sync.dma_start(
                t["xT"][:], xT_d.ap()[:, qt0:qt0 + QT].rearrange(
                    "(kb p) t -> p kb t", p=128))
            nc.sync.dma_start(
                t["xh"][:], xh_d.ap()[qt0:qt0 + QT, :].rearrange(
                    "(tt p) j -> p tt j", p=128))

        # ---------- P1: v-proj + rk for token-tile n ----------
        def p1_slice(qq, n):
            t = qtiles(qq)
            vps = ps.tile([128, d], F32, tag="bigp", bufs=2, name="vps")
            for kb in range(8):
                nc.tensor.matmul(vps[:], t["xT"][:, kb, n * 128:(n + 1) * 128],
                                 wwt_s[:, kb, :], start=(kb == 0), stop=(kb == 7))
            nc.scalar.activation(t["v_nat"][:, n, :], vps[:], COPY)
            sq = scr.tile([128, d], F32, tag="sq", name="sq")
            if os.environ.get("K_TTR", "0") == "1":
                nc.vector.tensor_tensor_reduce(
                    sq[:], t["xh"][:, n, :], t["xh"][:, n, :], 1.0, 0.0,
                    MULT, ADD, accum_out=t["nrm2"][:, n:n + 1])
            else:
                nc.scalar.activation(sq[:], t["xh"][:, n, :],
                                     mybir.ActivationFunctionType.Square,
                                     accum_out=t["nrm2"][:, n:n + 1])
            nc.scalar.activation(t["nrm"][:, n:n + 1], t["nrm2"][:, n:n + 1], SQRT)
            nc.vector.reciprocal(t["inv"][:, n:n + 1], t["nrm"][:, n:n + 1])
            GP.tensor_scalar(t["rk"][:, n, :], t["xh"][:, n, :],
                             t["inv"][:, n:n + 1], None, MULT)
            tp = ps.tile([128, 2, 128], BF16, tag="cgtp", bufs=2, name="tp")
            for kt in range(2):
                nc.tensor.transpose(tp[:, kt, :],
                                    t["rk"][:, n, kt * 128:(kt + 1) * 128], id_s[:])
            nc.vector.tensor_copy(
                t["rkT"][:, :, 1 + n * 128:1 + (n + 1) * 128], tp[:])

        # ---------- dprep: shift/scale prep for quarter qq ----------
        def dprep(qq):
            t = qtiles(qq)
            if qq == 0:
                nc.gpsimd.memset(t["rkT"][:, :, 0:1], 0.0)
                nc.gpsimd.memset(t["wk"][0:1, 0:1, :], 0.0)
            else:
                tprev = QS[qq - 1]
                nc.vector.tensor_copy(t["rkT"][:, :, 0:1],
                                      tprev["rkT"][:, :, QT:QT + 1])
                nc.sync.dma_start(t["wk"][0:1, 0:1, :],
                                  tprev["rk"][127:128, 7:8, :])
            nc.sync.dma_start(t["wk"][1:128, :, :], t["rk"][0:127, :, :])
            nc.sync.dma_start(t["wk"][0:1, 1:8, :], t["rk"][127:128, 0:7, :])
            GP.tensor_scalar(t["wkgN"][:], t["wk"][:], gpt_s[:, 0:1],
                             None, MULT)
            for kt in range(2):
                GP.tensor_mul(t["rkgT"][:, kt, :],
                              t["rkT"][:, kt, 1:QT + 1], gpb_s[:])

        # ---------- P2: chain, pair-batched (called per slot) ----------
        # pair state carried between even/odd slots
        pair = {}

        def p2_slot(qq, n):
            t = qtiles(qq)
            half = n % 2
            w0 = n * C
            if half == 0:
                pair["g"] = ps.tile([128, 2, 2, 128], F32, tag="g", bufs=2,
                                    name="gpair")
            g = pair["g"]
            for kt in range(2):
                nc.tensor.matmul(g[:, half, 0, :], t["rkT"][:, kt, w0:w0 + 128],
                                 t["rkT"][:, kt, w0:w0 + 128],
                                 start=(kt == 0), stop=(kt == 1))
            for kt in range(2):
                nc.tensor.matmul(g[:, half, 1, :], t["rkT"][:, kt, w0:w0 + 128],
                                 t["rkT"][:, kt, w0 + 1:w0 + 129],
                                 start=(kt == 0), stop=(kt == 1))
            if half == 0:
                return
            # odd slot: drains + chain for the pair (chunks n-1, n)
            p0 = n - 1
            B0 = chp.tile([128, 2, 128], BF16, tag="B0", name="B0")
            nc.vector.tensor_mul(B0[:], g[:, :, 0, :], mb2_s[:])
            C0 = chp.tile([128, 2, 128], BF16, tag="C0", name="C0")
            nc.vector.tensor_mul(C0[:], g[:, :, 0, :], mc2_s[:])
            nc.vector.tensor_mul(t["inT"][:, p0:p0 + 2, :], g[:, :, 1, :],
                                 mit2_s[:])
            G0 = chp.tile([128, 2, 128], BF16, tag="G0", name="G0")
            GP.tensor_add(G0[:], B0[:], id2_s[:])
            Gh0 = chp.tile([128, 2, 128], BF16, tag="Gh0", name="Gh0")
            GP.tensor_add(Gh0[:], C0[:], id2_s[:])
            c1p = ps.tile([128, 2, 128], F32, tag="cgtp", bufs=2, name="c1p")
            for ch in range(2):
                nc.tensor.matmul(c1p[:, ch, :], B0[:, ch, :], C0[:, ch, :])
            C1 = chp.tile([128, 2, 128], BF16, tag="C1", name="C1")
            nc.vector.tensor_copy(C1[:], c1p[:])
            g1p = ps.tile([128, 2, 128], F32, tag="cgtp", bufs=2, name="g1p")
            for ch in range(2):
                nc.tensor.matmul(g1p[:, ch, :], G0[:, ch, :], C1[:, ch, :])
            nc.vector.tensor_add(t["AT"][:, p0:p0 + 2, :], g1p[:], Gh0[:])
            wcp = ps.tile([128, 2, 2, 128], F32, tag="g", bufs=2, name="wcp")
            for ch in range(2):
                for jb in range(2):
                    nc.tensor.matmul(
                        wcp[:, ch, jb, :],
                        t["wk"][:, p0 + ch, jb * 128:(jb + 1) * 128],
                        t["AT"][:, p0 + ch, :])
            # negated store: wkcT = -(A wk)^T
            nc.scalar.activation(
                t["wkcT"][:, :, w0 - C:w0 + C].rearrange("p jb (ch c) -> p ch jb c", ch=2),
                wcp[:], COPY, scale=-1.0)

        # ---------- P3: scan chunk (jv-split chains) ----------
        def p3_chunk(qq, n):
            t = qtiles(qq)
            w0 = n * C
            vnp = ps.tile([128, 2, 128], F32, tag="scan", bufs=2, name="vnp")
            for jv in range(2):
                nc.tensor.matmul(vnp[:, jv, :], t["AT"][:, n, :],
                                 t["v_nat"][:, n, jv * 128:(jv + 1) * 128],
                                 start=True, stop=False)
                for jb in range(2):
                    nc.tensor.matmul(vnp[:, jv, :], t["wkcT"][:, jb, w0:w0 + 128],
                                     S_bf[:, jb, jv * 128:(jv + 1) * 128],
                                     start=False, stop=(jb == 1))
            vnew = vnw.tile([128, d], BF16, tag="vnew", name="vnew")
            nc.vector.tensor_copy(vnew[:, 0:128], vnp[:, 0, :])
            nc.scalar.activation(vnew[:, 128:256], vnp[:, 1, :], COPY)
            ot = ps.tile([128, 2, 128], F32, tag="scan", bufs=2, name="ot")
            for jv in range(2):
                for jb in range(2):
                    nc.tensor.matmul(ot[:, jv, :],
                                     S_bf[:, jb, jv * 128:(jv + 1) * 128],
                                     t["rkgT"][:, jb, w0:w0 + 128],
                                     start=(jb == 0), stop=False)
                nc.tensor.matmul(ot[:, jv, :],
                                 vnew[:, jv * 128:(jv + 1) * 128],
                                 t["inT"][:, n, :], start=False, stop=True)
            sup = ps.tile([128, 2, d], F32, tag="scan", bufs=2, name="sup")
            for jv in range(2):
                for jb in range(2):
                    nc.tensor.matmul(sup[:, jb, jv * 128:(jv + 1) * 128],
                                     gcvid_s[:], S_bf[:, jb, jv * 128:(jv + 1) * 128],
                                     start=True, stop=False)
                    nc.tensor.matmul(sup[:, jb, jv * 128:(jv + 1) * 128],
                                     t["wkgN"][:, n, jb * 128:(jb + 1) * 128],
                                     vnew[:, jv * 128:(jv + 1) * 128],
                                     start=False, stop=True)
            # S <- sup (gcv*S folded into matmul group)
            nc.vector.tensor_copy(S_bf[:, :, 0:128], sup[:, :, 0:128])
            nc.scalar.activation(S_bf[:, :, 128:256], sup[:, :, 128:256], COPY)
            nc.scalar.activation(t["oT"][:, :, w0:w0 + 128], ot[:], COPY)

        # ---------- P4: out-projection ----------
        def p4_chunk(qq, n, st):
            t = qtiles(qq)
            for nh in range(2):
                pps = ps.tile([128, 512], F32, tag="bigp", bufs=2, name="pps")
                for kt in range(2):
                    nc.tensor.matmul(pps[:], t["oT"][:, kt, n * 128:(n + 1) * 128],
                                     wrt_s[:, kt, nh * 512:(nh + 1) * 512],
                                     start=(kt == 0), stop=(kt == 1))
                if nh == 0:
                    nc.vector.tensor_copy(st[:, n % 2, 0:512], pps[:])
                else:
                    nc.scalar.activation(st[:, n % 2, 512:1024], pps[:], COPY)
            if n % 2 == 1:
                roff = qq * QT + (n - 1) * 128
                nc.sync.dma_start(
                    part_d.ap()[roff:roff + 256, :].rearrange(
                        "(c p) j -> p c j", p=128), st[:])

        # ---------------- schedule ----------------
        loads(0)
        for n in range(NCH):
            p1_slice(0, n)
        dprep(0)
        for q in range(NQ):
            if q < NQ - 1:
                loads(q + 1)
            st = None
            for n in range(10):
                if n < 8:
                    p2_slot(q, n)
                if 2 <= n:
                    p3_chunk(q, n - 2)
                if q > 0 and n < 8:
                    if n % 2 == 0:
                        st = st_p.tile([128, 2, QT], BF16, tag="st", name="st")
                    p4_chunk(q - 1, n, st)
                if q < NQ - 1 and n < 8:
                    p1_slice(q + 1, n)
            if q < NQ - 1:
                dprep(q + 1)
            if q >= 1:
                QS.pop(q - 1, None)
        for n in range(NCH):
            if n % 2 == 0:
                st = st_p.tile([128, 2, QT], BF16, tag="st", name="st")
            p4_chunk(NQ - 1, n, st)
    nc.compile()
    return nc


_NC = None
LAST_EXEC_NS = None
LAST_TRACE = None


def _bf16(a):
    return np.ascontiguousarray(np.asarray(a).astype(ml_dtypes.bfloat16))


def kernel(out, Ww, Wr, decay, log_alpha):
    global _NC
    out = np.asarray(out, dtype=np.float32)
    Ww = np.asarray(Ww, dtype=np.float32)
    Wr = np.asarray(Wr, dtype=np.float32)
    decay = np.asarray(decay, dtype=np.float32)
    log_alpha = np.asarray(log_alpha, dtype=np.float32)
    gamma = 1.0 / (1.0 + np.exp(-decay.astype(np.float64)))
    alpha = np.exp(log_alpha.astype(np.float64))

    if _NC is None:
        _NC = _build()
    nc = _NC

    pc = np.arange(C)
    xT_b = [_bf16(out[b].T) for b in range(B)]
    in_maps = []
    for ci in range(8):
        b, h = ci // 4, ci % 4
        g = gamma[h]
        Ls = np.tril(g ** np.maximum(pc[:, None] - pc[None, :], 0), -1)
        mb = (-Ls).astype(np.float32)
        mit = np.triu(g ** np.maximum(pc[None, :] - pc[:, None], 0), 1).astype(np.float32)
        gp = (g ** (np.arange(QT) % C)).astype(np.float32)
        gpb = np.broadcast_to(gp[None, :], (128, QT))
        gpt = (g ** (C - 1 - np.arange(128)))[:, None].astype(np.float32)
        in_maps.append({
            "xt": xT_b[b],
            "xh": _bf16(out[b][:, h * d:(h + 1) * d]),
            "wwt": _bf16(Ww[h * d:(h + 1) * d, :].T),
            "wrt": _bf16((alpha[h] * Wr[:, h * d:(h + 1) * d]).T),
            "mb": mb, "mc": np.ascontiguousarray(mb.T),
            "mit": mit,
            "ident": _bf16(np.eye(128, dtype=np.float32)),
            "gcvid": _bf16((g ** C) * np.eye(128, dtype=np.float32)),
            "gpbf": _bf16(gpb),
            "gpt": gpt,
        })

    ncore = int(os.environ.get("K_NCORES", "8"))
    res = bass_utils.run_bass_kernel_spmd(
        nc, in_maps[:ncore], core_ids=list(range(ncore)),
        trace=bool(os.environ.get("K_TRACE")))
    global LAST_EXEC_NS, LAST_TRACE
    LAST_EXEC_NS = res.exec_time_ns
    LAST_TRACE = res.instructions_and_trace
    final = out.copy()
    for ci in range(len(res.results)):
        b = ci // 4
        final[b] += res.results[ci]["partial"].astype(np.float32)
    return final


# revision 25
# speedup vs baseline: 2.5206x; 1.0222x over previous
"""DeltaHebbianBlock Trainium2 kernel (v3).

Sharding: 8 cores = (B=2) x (H=4). Each core runs its head's delta-rule
chunked scan (C=128, degree-3 UT chain) and the partial output projection
partial_bh = (alpha_h * o_bh) @ Wr_h^T. Host: out[b] = x[b] + sum_h partial.

Fully fused slot schedule per quarter (QT=1024, 8 chunks of C=128):
slot n runs P2(q,n) chain (pair-batched drains), P3(q,n-2) scan,
P4(q-1,n) out-proj, P1(q+1,n) v-proj/rk.  Tricks: wkcT stored negated so
vnew accumulates fully in PSUM (no TSP); S decay folded into the sup
matmul group via gcv*I; scan emits oT directly (no o transposes).
"""
import os
import numpy as np
import ml_dtypes
from contextlib import ExitStack

import concourse.bass as bass
import concourse.mybir as mybir
import concourse.tile as tile
from concourse import bacc, bass_utils

B, T, D = 2, 8192, 1024
H, d, C = 4, 256, 128
NQ = 8                # quarter passes
QT = T // NQ          # 1024 tokens per pass
NCH = QT // C         # 8 chunks per pass

F32 = mybir.dt.float32
BF16 = mybir.dt.bfloat16
F8 = mybir.dt.float8e4
MULT = mybir.AluOpType.mult
ADD = mybir.AluOpType.add
ACT_COPY = None  # set in _build


def _build():
    nc = bacc.Bacc("TRN2", target_bir_lowering=False, debug=False,
                   num_devices=int(os.environ.get("K_NCORES", "8")))
    xT_d = nc.dram_tensor("xt", (D, T), F8, kind="ExternalInput")
    xh_d = nc.dram_tensor("xh", (T, d), BF16, kind="ExternalInput")
    wwt_d = nc.dram_tensor("wwt", (D, d), F8, kind="ExternalInput")
    wrt_d = nc.dram_tensor("wrt", (d, D), F8, kind="ExternalInput")
    alsc_d = nc.dram_tensor("alsc", (128, 1), F32, kind="ExternalInput")
    mb_d = nc.dram_tensor("mb", (C, C), F32, kind="ExternalInput")
    mc_d = nc.dram_tensor("mc", (C, C), F32, kind="ExternalInput")
    mit_d = nc.dram_tensor("mit", (C, C), F32, kind="ExternalInput")
    id_d = nc.dram_tensor("ident", (128, 128), BF16, kind="ExternalInput")
    gcvid_d = nc.dram_tensor("gcvid", (128, 128), BF16, kind="ExternalInput")
    gpb_d = nc.dram_tensor("gpbf", (128, QT), BF16, kind="ExternalInput")
    gpt_d = nc.dram_tensor("gpt", (128, 1), F32, kind="ExternalInput")
    part_d = nc.dram_tensor("partial", (T, D), BF16, kind="ExternalOutput")

    COPY = mybir.ActivationFunctionType.Copy
    SQRT = mybir.ActivationFunctionType.Sqrt
    GP = nc.gpsimd if os.environ.get("K_POOL", "1") == "1" else nc.vector

    with ExitStack() as ctx:
        tc = ctx.enter_context(tile.TileContext(nc))
        consts = ctx.enter_context(tc.tile_pool(name="consts", bufs=1))
        qx = ctx.enter_context(tc.tile_pool(name="qx", bufs=2))
        qa = ctx.enter_context(tc.tile_pool(name="qa", bufs=2))
        chp = ctx.enter_context(tc.tile_pool(name="chp", bufs=2))
        vnw = ctx.enter_context(tc.tile_pool(name="vnw", bufs=3))
        st_p = ctx.enter_context(tc.tile_pool(name="stp", bufs=2))
        scr = ctx.enter_context(tc.tile_pool(name="scr", bufs=2))
        ps = ctx.enter_context(tc.tile_pool(name="ps", bufs=1, space="PSUM"))

        # ---- constants / weights in SBUF ----
        wwt_s = consts.tile([128, 8, d], F8)
        nc.sync.dma_start(wwt_s[:], wwt_d.ap().rearrange("(kb p) j -> p kb j", p=128))
        wrt_s = consts.tile([128, 2, D], F8)
        nc.sync.dma_start(wrt_s[:], wrt_d.ap().rearrange("(kt p) n -> p kt n", p=128))
        mb2_s = consts.tile([128, 2, 128], F32)
        mc2_s = consts.tile([128, 2, 128], F32)
        mit2_s = consts.tile([128, 2, 128], F32)
        id2_s = consts.tile([128, 2, 128], BF16)
        for ch in range(2):
            nc.sync.dma_start(mb2_s[:, ch, :], mb_d.ap())
            nc.sync.dma_start(mc2_s[:, ch, :], mc_d.ap())
            nc.sync.dma_start(mit2_s[:, ch, :], mit_d.ap())
            nc.sync.dma_start(id2_s[:, ch, :], id_d.ap())
        id_s = consts.tile([128, 128], BF16)
        nc.sync.dma_start(id_s[:], id_d.ap())
        gcvid_s = consts.tile([128, 128], BF16)
        nc.sync.dma_start(gcvid_s[:], gcvid_d.ap())
        gpb_s = consts.tile([128, QT], BF16)
        nc.sync.dma_start(gpb_s[:], gpb_d.ap())
        gpt_s = consts.tile([128, 1], F32)
        nc.sync.dma_start(gpt_s[:], gpt_d.ap())
        alsc_s = consts.tile([128, 1], F32)
        nc.sync.dma_start(alsc_s[:], alsc_d.ap())

        S_bf = consts.tile([128, 2, d], BF16)
        nc.gpsimd.memset(S_bf[:], 0.0)

        QS = {}  # per-quarter tile sets

        def qtiles(qq):
            if qq in QS:
                return QS[qq]
            t = {}
            t["xT"] = qx.tile([128, 8, QT], F8, tag="xT", name="xT")
            t["xh"] = qa.tile([128, 8, d], BF16, tag="xh", name="xh")
            t["v_nat"] = qa.tile([128, 8, d], BF16, tag="v_nat", name="v_nat")
            t["nrm2"] = scr.tile([128, 8], F32, tag="nrm2", name="nrm2")
            t["nrm"] = scr.tile([128, 8], F32, tag="nrm", name="nrm")
            t["inv"] = scr.tile([128, 8], F32, tag="inv", name="inv")
            t["rk"] = qa.tile([128, 8, d], BF16, tag="rk", name="rk")
            t["wk"] = qa.tile([128, 8, d], BF16, tag="wk", name="wk")
            t["wkgN"] = qa.tile([128, 8, d], BF16, tag="wkgN", name="wkgN")
            t["rkT"] = qa.tile([128, 2, QT + 1], BF16, tag="rkT", name="rkT")
            t["rkgT"] = qa.tile([128, 2, QT], BF16, tag="rkgT", name="rkgT")
            t["wkcT"] = qa.tile([128, 2, QT], BF16, tag="wkcT", name="wkcT")
            t["inT"] = qa.tile([128, NCH, C], BF16, tag="inT", name="inT")
            t["AT"] = qa.tile([128, NCH, C], BF16, tag="AT", name="AT")
            t["oT"] = qa.tile([128, 2, QT], F8, tag="oT", name="oT")
            QS[qq] = t
            return t

        def loads(qq):
            t = qtiles(qq)
            qt0 = qq * QT
            nc.sync.dma_start(
                t["xT"][:], xT_d.ap()[:, qt0:qt0 + QT].rearrange(
                    "(kb p) t -> p kb t", p=128))
            nc.sync.dma_start(
                t["xh"][:], xh_d.ap()[qt0:qt0 + QT, :].rearrange(
                    "(tt p) j -> p tt j", p=128))

        # ---------- P1: v-proj + rk for token-tile n ----------
        def p1_slice(qq, n):
            t = qtiles(qq)
            vps = ps.tile([128, d], F32, tag="bigp", bufs=2, name="vps")
            for kp in range(4):
                nc.tensor.matmul(vps[:], t["xT"][:, 2 * kp:2 * kp + 2, n * 128:(n + 1) * 128],
                                 wwt_s[:, 2 * kp:2 * kp + 2, :], start=(kp == 0), stop=(kp == 3),
                                 perf_mode=mybir.MatmulPerfMode.DoubleRow)
            nc.scalar.activation(t["v_nat"][:, n, :], vps[:], COPY)
            sq = scr.tile([128, d], F32, tag="sq", name="sq")
            nc.scalar.activation(sq[:], t["xh"][:, n, :],
                                 mybir.ActivationFunctionType.Square,
                                 accum_out=t["nrm2"][:, n:n + 1])
            nc.scalar.activation(t["nrm"][:, n:n + 1], t["nrm2"][:, n:n + 1], SQRT)
            nc.vector.reciprocal(t["inv"][:, n:n + 1], t["nrm"][:, n:n + 1])
            GP.tensor_scalar(t["rk"][:, n, :], t["xh"][:, n, :],
                             t["inv"][:, n:n + 1], None, MULT)
            tp = ps.tile([128, 2, 128], BF16, tag="cgtp", bufs=2, name="tp")
            for kt in range(2):
                nc.tensor.transpose(tp[:, kt, :],
                                    t["rk"][:, n, kt * 128:(kt + 1) * 128], id_s[:])
            nc.vector.tensor_copy(
                t["rkT"][:, :, 1 + n * 128:1 + (n + 1) * 128], tp[:])

        # ---------- dprep: shift/scale prep for quarter qq ----------
        def dprep(qq):
            t = qtiles(qq)
            if qq == 0:
                nc.gpsimd.memset(t["rkT"][:, :, 0:1], 0.0)
                nc.gpsimd.memset(t["wk"][0:1, 0:1, :], 0.0)
            else:
                tprev = QS[qq - 1]
                nc.vector.tensor_copy(t["rkT"][:, :, 0:1],
                                      tprev["rkT"][:, :, QT:QT + 1])
                nc.sync.dma_start(t["wk"][0:1, 0:1, :],
                                  tprev["rk"][127:128, 7:8, :])
            nc.sync.dma_start(t["wk"][1:128, :, :], t["rk"][0:127, :, :])
            nc.sync.dma_start(t["wk"][0:1, 1:8, :], t["rk"][127:128, 0:7, :])
            GP.tensor_scalar(t["wkgN"][:], t["wk"][:], gpt_s[:, 0:1],
                             None, MULT)
            for kt in range(2):
                GP.tensor_mul(t["rkgT"][:, kt, :],
                              t["rkT"][:, kt, 1:QT + 1], gpb_s[:])

        # ---------- P2: chain, pair-batched (called per slot) ----------
        # pair state carried between even/odd slots
        pair = {}

        def p2_slot(qq, n):
            t = qtiles(qq)
            half = n % 2
            w0 = n * C
            if half == 0:
                pair["g"] = ps.tile([128, 2, 2, 128], F32, tag="g", bufs=2,
                                    name="gpair")
            g = pair["g"]
            for kt in range(2):
                nc.tensor.matmul(g[:, half, 0, :], t["rkT"][:, kt, w0:w0 + 128],
                                 t["rkT"][:, kt, w0:w0 + 128],
                                 start=(kt == 0), stop=(kt == 1))
            for kt in range(2):
                nc.tensor.matmul(g[:, half, 1, :], t["rkT"][:, kt, w0:w0 + 128],
                                 t["rkT"][:, kt, w0 + 1:w0 + 129],
                                 start=(kt == 0), stop=(kt == 1))
            if half == 0:
                return
            # odd slot: drains + chain for the pair (chunks n-1, n)
            p0 = n - 1
            B0 = chp.tile([128, 2, 128], BF16, tag="B0", name="B0")
            nc.vector.tensor_mul(B0[:], g[:, :, 0, :], mb2_s[:])
            C0 = chp.tile([128, 2, 128], BF16, tag="C0", name="C0")
            nc.vector.tensor_mul(C0[:], g[:, :, 0, :], mc2_s[:])
            nc.vector.tensor_mul(t["inT"][:, p0:p0 + 2, :], g[:, :, 1, :],
                                 mit2_s[:])
            G0 = chp.tile([128, 2, 128], BF16, tag="G0", name="G0")
            GP.tensor_add(G0[:], B0[:], id2_s[:])
            Gh0 = chp.tile([128, 2, 128], BF16, tag="Gh0", name="Gh0")
            GP.tensor_add(Gh0[:], C0[:], id2_s[:])
            c1p = ps.tile([128, 2, 128], F32, tag="cgtp", bufs=2, name="c1p")
            for ch in range(2):
                nc.tensor.matmul(c1p[:, ch, :], B0[:, ch, :], C0[:, ch, :])
            C1 = chp.tile([128, 2, 128], BF16, tag="C1", name="C1")
            nc.vector.tensor_copy(C1[:], c1p[:])
            g1p = ps.tile([128, 2, 128], F32, tag="cgtp", bufs=2, name="g1p")
            for ch in range(2):
                nc.tensor.matmul(g1p[:, ch, :], G0[:, ch, :], C1[:, ch, :])
            nc.vector.tensor_add(t["AT"][:, p0:p0 + 2, :], g1p[:], Gh0[:])
            wcp = ps.tile([128, 2, 2, 128], F32, tag="g", bufs=2, name="wcp")
            for ch in range(2):
                for jb in range(2):
                    nc.tensor.matmul(
                        wcp[:, ch, jb, :],
                        t["wk"][:, p0 + ch, jb * 128:(jb + 1) * 128],
                        t["AT"][:, p0 + ch, :])
            # negated store: wkcT = -(A wk)^T
            nc.scalar.activation(
                t["wkcT"][:, :, w0 - C:w0 + C].rearrange("p jb (ch c) -> p ch jb c", ch=2),
                wcp[:], COPY, scale=-1.0)

        # ---------- P3: scan chunk (jv-split chains) ----------
        def p3_chunk(qq, n):
            t = qtiles(qq)
            w0 = n * C
            vnp = ps.tile([128, 2, 128], F32, tag="scan", bufs=2, name="vnp")
            nc.tensor.matmul(vnp[:, :, :], t["AT"][:, n, :], t["v_nat"][:, n, :],
                             start=True, stop=False)
            for jv in range(2):
                for jb in range(2):
                    nc.tensor.matmul(vnp[:, jv, :], t["wkcT"][:, jb, w0:w0 + 128],
                                     S_bf[:, jb, jv * 128:(jv + 1) * 128],
                                     start=False, stop=(jv == 1 and jb == 1))
            vnew = vnw.tile([128, d], BF16, tag="vnew", name="vnew")
            nc.vector.tensor_copy(vnew[:, 0:128], vnp[:, 0, :])
            nc.scalar.activation(vnew[:, 128:256], vnp[:, 1, :], COPY)
            ot = ps.tile([128, 2, 128], F32, tag="scan", bufs=2, name="ot")
            for jv in range(2):
                for jb in range(2):
                    nc.tensor.matmul(ot[:, jv, :],
                                     S_bf[:, jb, jv * 128:(jv + 1) * 128],
                                     t["rkgT"][:, jb, w0:w0 + 128],
                                     start=(jb == 0), stop=False)
                nc.tensor.matmul(ot[:, jv, :],
                                 vnew[:, jv * 128:(jv + 1) * 128],
                                 t["inT"][:, n, :], start=False, stop=True)
            sup = ps.tile([128, 2, d], F32, tag="scan", bufs=2, name="sup")
            for jb in range(2):
                nc.tensor.matmul(sup[:, jb, :], gcvid_s[:], S_bf[:, jb, :],
                                 start=True, stop=False)
                for jv in range(2):
                    nc.tensor.matmul(sup[:, jb, jv * 128:(jv + 1) * 128],
                                     t["wkgN"][:, n, jb * 128:(jb + 1) * 128],
                                     vnew[:, jv * 128:(jv + 1) * 128],
                                     start=False, stop=(jv == 1))
            # S <- sup (gcv*S folded into matmul group)
            nc.vector.tensor_copy(S_bf[:, :, 0:128], sup[:, :, 0:128])
            nc.scalar.activation(S_bf[:, :, 128:256], sup[:, :, 128:256], COPY)
            nc.scalar.activation(t["oT"][:, :, w0:w0 + 128], ot[:], COPY,
                                 scale=alsc_s[:, 0:1])

        # ---------- P4: out-projection ----------
        def p4_chunk(qq, n, st):
            t = qtiles(qq)
            for nh in range(2):
                pps = ps.tile([128, 512], F32, tag="bigp", bufs=2, name="pps")
                nc.tensor.matmul(pps[:], t["oT"][:, :, n * 128:(n + 1) * 128],
                                 wrt_s[:, :, nh * 512:(nh + 1) * 512],
                                 start=True, stop=True,
                                 perf_mode=mybir.MatmulPerfMode.DoubleRow)
                if nh == 0:
                    nc.vector.tensor_copy(st[:, n % 2, 0:512], pps[:])
                else:
                    nc.scalar.activation(st[:, n % 2, 512:1024], pps[:], COPY)
            if n % 2 == 1:
                roff = qq * QT + (n - 1) * 128
                nc.sync.dma_start(
                    part_d.ap()[roff:roff + 256, :].rearrange(
                        "(c p) j -> p c j", p=128), st[:])

        # ---------------- schedule ----------------
        loads(0)
        for n in range(NCH):
            p1_slice(0, n)
        dprep(0)
        for q in range(NQ):
            if q < NQ - 1:
                loads(q + 1)
            st = None
            for n in range(10):
                if n < 8:
                    p2_slot(q, n)
                if 2 <= n:
                    p3_chunk(q, n - 2)
                if q > 0 and n < 8:
                    if n % 2 == 0:
                        st = st_p.tile([128, 2, QT], BF16, tag="st", name="st")
                    p4_chunk(q - 1, n, st)
                if q < NQ - 1 and n < 8:
                    p1_slice(q + 1, n)
            if q < NQ - 1:
                dprep(q + 1)
            if q >= 1:
                QS.pop(q - 1, None)
        for n in range(NCH):
            if n % 2 == 0:
                st = st_p.tile([128, 2, QT], BF16, tag="st", name="st")
            p4_chunk(NQ - 1, n, st)
    nc.compile()
    return nc


_NC = None
LAST_EXEC_NS = None
LAST_TRACE = None


def _bf16(a):
    return np.ascontiguousarray(np.asarray(a).astype(ml_dtypes.bfloat16))


def _f8(a):
    return np.ascontiguousarray(np.asarray(a).astype(ml_dtypes.float8_e4m3))


def kernel(out, Ww, Wr, decay, log_alpha):
    global _NC
    out = np.asarray(out, dtype=np.float32)
    Ww = np.asarray(Ww, dtype=np.float32)
    Wr = np.asarray(Wr, dtype=np.float32)
    decay = np.asarray(decay, dtype=np.float32)
    log_alpha = np.asarray(log_alpha, dtype=np.float32)
    gamma = 1.0 / (1.0 + np.exp(-decay.astype(np.float64)))
    alpha = np.exp(log_alpha.astype(np.float64))

    if _NC is None:
        _NC = _build()
    nc = _NC

    pc = np.arange(C)
    xT_b = [_f8(out[b].T) for b in range(B)]
    in_maps = []
    for ci in range(8):
        b, h = ci // 4, ci % 4
        g = gamma[h]
        Ls = np.tril(g ** np.maximum(pc[:, None] - pc[None, :], 0), -1)
        mb = (-Ls).astype(np.float32)
        mit = np.triu(g ** np.maximum(pc[None, :] - pc[:, None], 0), 1).astype(np.float32)
        gp = (g ** (np.arange(QT) % C)).astype(np.float32)
        gpb = np.broadcast_to(gp[None, :], (128, QT))
        gpt = (g ** (C - 1 - np.arange(128)))[:, None].astype(np.float32)
        in_maps.append({
            "xt": xT_b[b],
            "xh": _bf16(out[b][:, h * d:(h + 1) * d]),
            "wwt": _f8(Ww[h * d:(h + 1) * d, :].T),
            "wrt": _f8(Wr[:, h * d:(h + 1) * d].T),
            "alsc": np.full((128, 1), alpha[h], np.float32),
            "mb": mb, "mc": np.ascontiguousarray(mb.T),
            "mit": mit,
            "ident": _bf16(np.eye(128, dtype=np.float32)),
            "gcvid": _bf16((g ** C) * np.eye(128, dtype=np.float32)),
            "gpbf": _bf16(gpb),
            "gpt": gpt,
        })

    ncore = int(os.environ.get("K_NCORES", "8"))
    res = bass_utils.run_bass_kernel_spmd(
        nc, in_maps[:ncore], core_ids=list(range(ncore)),
        trace=bool(os.environ.get("K_TRACE")))
    global LAST_EXEC_NS, LAST_TRACE
    LAST_EXEC_NS = res.exec_time_ns
    LAST_TRACE = res.instructions_and_trace
    final = out.copy()
    for ci in range(len(res.results)):
        b = ci // 4
        final[b] += res.results[ci]["partial"].astype(np.float32)
    return final


# revision 39
# speedup vs baseline: 2.7364x; 1.0856x over previous
"""DeltaHebbianBlock Trainium2 kernel (v3).

Sharding: 8 cores = (B=2) x (H=4). Each core runs its head's delta-rule
chunked scan (C=128, degree-3 UT chain) and the partial output projection
partial_bh = (alpha_h * o_bh) @ Wr_h^T. Host: out[b] = x[b] + sum_h partial.

Fully fused slot schedule per quarter (QT=1024, 8 chunks of C=128):
slot n runs P2(q,n) chain (pair-batched drains), P3(q,n-2) scan,
P4(q-1,n) out-proj, P1(q+1,n) v-proj/rk.  Tricks: wkcT stored negated so
vnew accumulates fully in PSUM (no TSP); S decay folded into the sup
matmul group via gcv*I; scan emits oT directly (no o transposes).
"""
import os
import numpy as np
import ml_dtypes
from contextlib import ExitStack

import concourse.bass as bass
import concourse.mybir as mybir
import concourse.tile as tile
from concourse import bacc, bass_utils

B, T, D = 2, 8192, 1024
H, d, C = 4, 256, 128
NQ = 8                # quarter passes
QT = T // NQ          # 1024 tokens per pass
NCH = QT // C         # 8 chunks per pass

F32 = mybir.dt.float32
BF16 = mybir.dt.bfloat16
F8 = mybir.dt.float8e4
MULT = mybir.AluOpType.mult
ADD = mybir.AluOpType.add
ACT_COPY = None  # set in _build


def _build():
    nc = bacc.Bacc("TRN2", target_bir_lowering=False, debug=False,
                   num_devices=int(os.environ.get("K_NCORES", "8")))
    xT_d = nc.dram_tensor("xt", (D, T), F8, kind="ExternalInput")
    xh_d = nc.dram_tensor("xh", (T, d), BF16, kind="ExternalInput")
    wwt_d = nc.dram_tensor("wwt", (D, d), F8, kind="ExternalInput")
    wrt_d = nc.dram_tensor("wrt", (d, D), F8, kind="ExternalInput")
    alsc_d = nc.dram_tensor("alsc", (128, 1), F32, kind="ExternalInput")
    mb_d = nc.dram_tensor("mb", (C, C), F32, kind="ExternalInput")
    mc_d = nc.dram_tensor("mc", (C, C), F32, kind="ExternalInput")
    mit_d = nc.dram_tensor("mit", (C, C), F32, kind="ExternalInput")
    id_d = nc.dram_tensor("ident", (128, 128), BF16, kind="ExternalInput")
    gcvid_d = nc.dram_tensor("gcvid", (128, 128), BF16, kind="ExternalInput")
    gpb_d = nc.dram_tensor("gpbf", (128, QT), BF16, kind="ExternalInput")
    gpt_d = nc.dram_tensor("gpt", (128, 1), F32, kind="ExternalInput")
    part_d = nc.dram_tensor("partial", (T, D), BF16, kind="ExternalOutput")

    COPY = mybir.ActivationFunctionType.Copy
    SQRT = mybir.ActivationFunctionType.Sqrt
    GP = nc.gpsimd if os.environ.get("K_POOL", "1") == "1" else nc.vector

    with ExitStack() as ctx:
        tc = ctx.enter_context(tile.TileContext(nc))
        consts = ctx.enter_context(tc.tile_pool(name="consts", bufs=1))
        qx = ctx.enter_context(tc.tile_pool(name="qx", bufs=2))
        qa = ctx.enter_context(tc.tile_pool(name="qa", bufs=2))
        chp = ctx.enter_context(tc.tile_pool(name="chp", bufs=2))
        vnw = ctx.enter_context(tc.tile_pool(name="vnw", bufs=3))
        st_p = ctx.enter_context(tc.tile_pool(name="stp", bufs=2))
        scr = ctx.enter_context(tc.tile_pool(name="scr", bufs=2))
        ps = ctx.enter_context(tc.tile_pool(name="ps", bufs=1, space="PSUM"))

        # ---- constants / weights in SBUF ----
        wwt_s = consts.tile([128, 8, d], F8)
        nc.sync.dma_start(wwt_s[:], wwt_d.ap().rearrange("(kb p) j -> p kb j", p=128))
        wrt_s = consts.tile([128, 2, D], F8)
        nc.sync.dma_start(wrt_s[:], wrt_d.ap().rearrange("(kt p) n -> p kt n", p=128))
        mb2_s = consts.tile([128, 2, 128], F32)
        mc2_s = consts.tile([128, 2, 128], F32)
        mit2_s = consts.tile([128, 2, 128], F32)
        id2_s = consts.tile([128, 2, 128], BF16)
        for ch in range(2):
            nc.sync.dma_start(mb2_s[:, ch, :], mb_d.ap())
            nc.sync.dma_start(mc2_s[:, ch, :], mc_d.ap())
            nc.sync.dma_start(mit2_s[:, ch, :], mit_d.ap())
            nc.sync.dma_start(id2_s[:, ch, :], id_d.ap())
        id_s = consts.tile([128, 128], BF16)
        nc.sync.dma_start(id_s[:], id_d.ap())
        gcvid_s = consts.tile([128, 128], BF16)
        nc.sync.dma_start(gcvid_s[:], gcvid_d.ap())
        gpb_s = consts.tile([128, QT], BF16)
        nc.sync.dma_start(gpb_s[:], gpb_d.ap())
        gpt_s = consts.tile([128, 1], F32)
        nc.sync.dma_start(gpt_s[:], gpt_d.ap())
        alsc_s = consts.tile([128, 1], F32)
        nc.sync.dma_start(alsc_s[:], alsc_d.ap())
        ones_s = consts.tile([128, 1], BF16)
        nc.gpsimd.memset(ones_s[:], 1.0)

        S_bf = consts.tile([128, 2, d], BF16)
        nc.gpsimd.memset(S_bf[:], 0.0)

        QS = {}  # per-quarter tile sets

        def qtiles(qq):
            if qq in QS:
                return QS[qq]
            t = {}
            t["xT"] = qx.tile([128, 8, QT], F8, tag="xT", name="xT")
            t["xh"] = qa.tile([128, 8, d], BF16, tag="xh", name="xh")
            t["v_nat"] = qa.tile([128, 8, d], BF16, tag="v_nat", name="v_nat")
            t["nrm2"] = scr.tile([128, 8], F32, tag="nrm2", name="nrm2")
            t["nrm"] = scr.tile([128, 8], F32, tag="nrm", name="nrm")
            t["inv"] = scr.tile([128, 8], F32, tag="inv", name="inv")
            t["rk"] = qa.tile([128, 8, d], BF16, tag="rk", name="rk")
            t["wk"] = qa.tile([128, 8, d], BF16, tag="wk", name="wk")
            t["wkgN"] = qa.tile([128, 8, d], BF16, tag="wkgN", name="wkgN")
            t["rkT"] = qa.tile([128, 2, QT + 1], BF16, tag="rkT", name="rkT")
            t["rkgT"] = qa.tile([128, 2, QT], BF16, tag="rkgT", name="rkgT")
            t["wkcT"] = qa.tile([128, 2, QT], BF16, tag="wkcT", name="wkcT")
            t["inT"] = qa.tile([128, NCH, C], BF16, tag="inT", name="inT")
            t["AT"] = qa.tile([128, NCH, C], BF16, tag="AT", name="AT")
            t["oT"] = qa.tile([128, 2, QT], F8, tag="oT", name="oT")
            QS[qq] = t
            return t

        def loads(qq):
            t = qtiles(qq)
            qt0 = qq * QT
            nc.sync.dma_start(
                t["xT"][:], xT_d.ap()[:, qt0:qt0 + QT].rearrange(
                    "(kb p) t -> p kb t", p=128))
            nc.sync.dma_start(
                t["xh"][:], xh_d.ap()[qt0:qt0 + QT, :].rearrange(
                    "(tt p) j -> p tt j", p=128))


        # ---------- P1: v-proj + rk for token-tile n ----------
        def p1_slice(qq, n):
            t = qtiles(qq)
            vps = ps.tile([128, d], F32, tag="bigp", bufs=2, name="vps")
            for kp in range(4):
                nc.tensor.matmul(vps[:], t["xT"][:, 2 * kp:2 * kp + 2, n * 128:(n + 1) * 128],
                                 wwt_s[:, 2 * kp:2 * kp + 2, :], start=(kp == 0), stop=(kp == 3),
                                 perf_mode=mybir.MatmulPerfMode.DoubleRow)
            nc.scalar.activation(t["v_nat"][:, n, :], vps[:], COPY)
            sq = scr.tile([128, d], F32, tag="sq", name="sq")
            nc.scalar.activation(sq[:], t["xh"][:, n, :],
                                 mybir.ActivationFunctionType.Square,
                                 accum_out=t["nrm2"][:, n:n + 1])
            nc.scalar.activation(t["nrm"][:, n:n + 1], t["nrm2"][:, n:n + 1], SQRT)
            nc.vector.reciprocal(t["inv"][:, n:n + 1], t["nrm"][:, n:n + 1])
            nc.gpsimd.tensor_scalar(t["rk"][:, n, :], t["xh"][:, n, :],
                                    t["inv"][:, n:n + 1], None, MULT)
            tp = ps.tile([128, 2, 128], BF16, tag="cgtp", bufs=2, name="tp")
            for kt in range(2):
                nc.tensor.transpose(tp[:, kt, :],
                                    t["rk"][:, n, kt * 128:(kt + 1) * 128], id_s[:])
            nc.vector.tensor_copy(
                t["rkT"][:, :, 1 + n * 128:1 + (n + 1) * 128], tp[:])

        # ---------- dprep: shift/scale prep for quarter qq ----------
        def dprep(qq):
            t = qtiles(qq)
            if qq == 0:
                nc.gpsimd.memset(t["rkT"][:, :, 0:1], 0.0)
                nc.gpsimd.memset(t["wk"][0:1, 0:1, :], 0.0)
            else:
                tprev = QS[qq - 1]
                nc.vector.tensor_copy(t["rkT"][:, :, 0:1],
                                      tprev["rkT"][:, :, QT:QT + 1])
                nc.sync.dma_start(t["wk"][0:1, 0:1, :],
                                  tprev["rk"][127:128, 7:8, :])
            nc.sync.dma_start(t["wk"][1:128, :, :], t["rk"][0:127, :, :])
            nc.sync.dma_start(t["wk"][0:1, 1:8, :], t["rk"][127:128, 0:7, :])
            nc.gpsimd.tensor_scalar(t["wkgN"][:], t["wk"][:], gpt_s[:, 0:1],
                                    None, MULT)
            for kt in range(2):
                nc.gpsimd.tensor_mul(t["rkgT"][:, kt, :],
                                     t["rkT"][:, kt, 1:QT + 1], gpb_s[:])

        # ---------- P2: chain, pair-batched (called per slot) ----------
        # pair state carried between even/odd slots
        pair = {}

        def p2_slot(qq, n):
            t = qtiles(qq)
            half = n % 2
            w0 = n * C
            if half == 0:
                pair["g"] = ps.tile([128, 2, 2, 128], F32, tag="g", bufs=2,
                                    name="gpair")
            g = pair["g"]
            for kt in range(2):
                nc.tensor.matmul(g[:, half, 0, :], t["rkT"][:, kt, w0:w0 + 128],
                                 t["rkT"][:, kt, w0:w0 + 128],
                                 start=(kt == 0), stop=(kt == 1))
            for kt in range(2):
                nc.tensor.matmul(g[:, half, 1, :], t["rkT"][:, kt, w0:w0 + 128],
                                 t["rkT"][:, kt, w0 + 1:w0 + 129],
                                 start=(kt == 0), stop=(kt == 1))
            if half == 0:
                return
            # odd slot: drains + chain for the pair (chunks n-1, n)
            p0 = n - 1
            B0 = chp.tile([128, 2, 128], BF16, tag="B0", name="B0")
            nc.vector.tensor_mul(B0[:], g[:, :, 0, :], mb2_s[:])
            C0 = chp.tile([128, 2, 128], BF16, tag="C0", name="C0")
            nc.vector.tensor_mul(C0[:], g[:, :, 0, :], mc2_s[:])
            nc.vector.tensor_mul(t["inT"][:, p0:p0 + 2, :], g[:, :, 1, :],
                                 mit2_s[:])
            c1p = ps.tile([128, 2, 128], F32, tag="cgtp", bufs=2, name="c1p")
            for ch in range(2):
                nc.tensor.matmul(c1p[:, ch, :], B0[:, ch, :], C0[:, ch, :])
            C1 = chp.tile([128, 2, 128], BF16, tag="C1", name="C1")
            nc.scalar.activation(C1[:], c1p[:], COPY)
            # g1p = (I + B0)^T C1 = C1 + C0 C1 ; AT' = g1p + C0 = A^T - I
            g1p = ps.tile([128, 2, 128], F32, tag="cgtp", bufs=2, name="g1p")
            for ch in range(2):
                nc.tensor.matmul(g1p[:, ch, :], id_s[:], C1[:, ch, :],
                                 start=True, stop=False)
                nc.tensor.matmul(g1p[:, ch, :], B0[:, ch, :], C1[:, ch, :],
                                 start=False, stop=True)
            nc.vector.tensor_add(t["AT"][:, p0:p0 + 2, :], g1p[:], C0[:])
            wcp = ps.tile([128, 2, 2, 128], F32, tag="g", bufs=2, name="wcp")
            for ch in range(2):
                for jb in range(2):
                    nc.tensor.matmul(
                        wcp[:, ch, jb, :],
                        t["wk"][:, p0 + ch, jb * 128:(jb + 1) * 128],
                        t["AT"][:, p0 + ch, :])
            # negated store: wkcT = -(A wk)^T = -(wcp + wk^T)
            for ch in range(2):
                c0 = w0 - C + ch * 128
                nc.vector.scalar_tensor_tensor(
                    t["wkcT"][:, :, c0:c0 + 128], wcp[:, ch, :, :], -1.0,
                    t["rkT"][:, :, c0:c0 + 128], MULT, mybir.AluOpType.subtract)

        # ---------- P3: scan chunk (jv-split chains) ----------
        def p3_chunk(qq, n):
            t = qtiles(qq)
            w0 = n * C
            vnp = ps.tile([128, 2, 128], F32, tag="scan", bufs=2, name="vnp")
            nc.tensor.matmul(vnp[:, :, :], id_s[:], t["v_nat"][:, n, :],
                             start=True, stop=False)
            nc.tensor.matmul(vnp[:, :, :], t["AT"][:, n, :], t["v_nat"][:, n, :],
                             start=False, stop=False)
            for jv in range(2):
                for jb in range(2):
                    nc.tensor.matmul(vnp[:, jv, :], t["wkcT"][:, jb, w0:w0 + 128],
                                     S_bf[:, jb, jv * 128:(jv + 1) * 128],
                                     start=False, stop=(jv == 1 and jb == 1))
            vnew = vnw.tile([128, d], BF16, tag="vnew", name="vnew")
            nc.vector.tensor_copy(vnew[:, 0:128], vnp[:, 0, :])
            nc.scalar.activation(vnew[:, 128:256], vnp[:, 1, :], COPY)
            ot = ps.tile([128, 2, 128], F32, tag="scan", bufs=2, name="ot")
            for jv in range(2):
                for jb in range(2):
                    nc.tensor.matmul(ot[:, jv, :],
                                     S_bf[:, jb, jv * 128:(jv + 1) * 128],
                                     t["rkgT"][:, jb, w0:w0 + 128],
                                     start=(jb == 0), stop=False)
                nc.tensor.matmul(ot[:, jv, :],
                                 vnew[:, jv * 128:(jv + 1) * 128],
                                 t["inT"][:, n, :], start=False, stop=True)
            sup = ps.tile([128, 2, d], F32, tag="scan", bufs=2, name="sup")
            for jb in range(2):
                nc.tensor.matmul(sup[:, jb, :], gcvid_s[:], S_bf[:, jb, :],
                                 start=True, stop=False)
                for jv in range(2):
                    nc.tensor.matmul(sup[:, jb, jv * 128:(jv + 1) * 128],
                                     t["wkgN"][:, n, jb * 128:(jb + 1) * 128],
                                     vnew[:, jv * 128:(jv + 1) * 128],
                                     start=False, stop=(jv == 1))
            # S <- sup (gcv*S folded into matmul group)
            nc.vector.tensor_copy(S_bf[:, :, 0:128], sup[:, :, 0:128])
            nc.scalar.activation(S_bf[:, :, 128:256], sup[:, :, 128:256], COPY)
            nc.scalar.activation(t["oT"][:, :, w0:w0 + 128], ot[:], COPY,
                                 scale=alsc_s[:, 0:1])

        # ---------- P4: out-projection ----------
        def p4_chunk(qq, n, st):
            t = qtiles(qq)
            for nh in range(2):
                pps = ps.tile([128, 512], F32, tag="bigp", bufs=2, name="pps")
                nc.tensor.matmul(pps[:], t["oT"][:, :, n * 128:(n + 1) * 128],
                                 wrt_s[:, :, nh * 512:(nh + 1) * 512],
                                 start=True, stop=True,
                                 perf_mode=mybir.MatmulPerfMode.DoubleRow)
                if nh == 0:
                    nc.vector.tensor_copy(st[:, n % 2, 0:512], pps[:])
                else:
                    nc.scalar.activation(st[:, n % 2, 512:1024], pps[:], COPY)
            if n % 2 == 1:
                roff = qq * QT + (n - 1) * 128
                nc.sync.dma_start(
                    part_d.ap()[roff:roff + 256, :].rearrange(
                        "(c p) j -> p c j", p=128), st[:])

        # ---------------- schedule ----------------
        loads(0)
        loads(1)
        for n in range(NCH):
            p1_slice(0, n)
        dprep(0)
        LOADS_AHEAD = True
        # P4(q, m) runs at slot m+4 of quarter q (oT(q, m) drained at m+2);
        # chunks 6,7 spill to slots 0,1 of the next quarter.
        p4q = []   # pending (qq, chunk) in order
        st_box = [None]

        def p4_push(qq, m):
            p4q.append((qq, m))

        def p4_pop():
            if not p4q:
                return
            qq, m = p4q.pop(0)
            if m % 2 == 0:
                st_box[0] = st_p.tile([128, 2, QT], BF16, tag="st", name="st")
            p4_chunk(qq, m, st_box[0])

        for q in range(NQ):
            if 2 <= q + 1 < NQ:
                loads(q + 1)
            for n in range(10):
                if n < 8:
                    p2_slot(q, n)
                if 2 <= n:
                    p3_chunk(q, n - 2)
                    p4_push(q, n - 2)
                LAG = int(os.environ.get("K_P4LAG", "6"))
                while p4q and (p4q[0][0] < q or n - p4q[0][1] >= LAG):
                    p4_pop()
                    break
                if q < NQ - 1 and n < 8:
                    p1_slice(q + 1, n)
            if q < NQ - 1:
                dprep(q + 1)
            if q >= 1:
                QS.pop(q - 1, None)
        while p4q:
            p4_pop()
    nc.compile()
    return nc


_NC = None
LAST_EXEC_NS = None
LAST_TRACE = None


def _bf16(a):
    return np.ascontiguousarray(np.asarray(a).astype(ml_dtypes.bfloat16))


def _f8(a):
    return np.ascontiguousarray(np.asarray(a).astype(ml_dtypes.float8_e4m3))


def kernel(out, Ww, Wr, decay, log_alpha):
    global _NC
    out = np.asarray(out, dtype=np.float32)
    Ww = np.asarray(Ww, dtype=np.float32)
    Wr = np.asarray(Wr, dtype=np.float32)
    decay = np.asarray(decay, dtype=np.float32)
    log_alpha = np.asarray(log_alpha, dtype=np.float32)
    gamma = 1.0 / (1.0 + np.exp(-decay.astype(np.float64)))
    alpha = np.exp(log_alpha.astype(np.float64))

    if _NC is None:
        _NC = _build()
    nc = _NC

    pc = np.arange(C)
    xT_b = [_f8(out[b].T) for b in range(B)]
    in_maps = []
    for ci in range(8):
        b, h = ci // 4, ci % 4
        g = gamma[h]
        Ls = np.tril(g ** np.maximum(pc[:, None] - pc[None, :], 0), -1)
        mb = (-Ls).astype(np.float32)
        mit = np.triu(g ** np.maximum(pc[None, :] - pc[:, None], 0), 1).astype(np.float32)
        gp = (g ** (np.arange(QT) % C)).astype(np.float32)
        gpb = np.broadcast_to(gp[None, :], (128, QT))
        gpt = (g ** (C - 1 - np.arange(128)))[:, None].astype(np.float32)
        in_maps.append({
            "xt": xT_b[b],
            "xh": _bf16(out[b][:, h * d:(h + 1) * d]),
            "wwt": _f8(Ww[h * d:(h + 1) * d, :].T),
            "wrt": _f8(Wr[:, h * d:(h + 1) * d].T),
            "alsc": np.full((128, 1), alpha[h], np.float32),
            "mb": mb, "mc": np.ascontiguousarray(mb.T),
            "mit": mit,
            "ident": _bf16(np.eye(128, dtype=np.float32)),
            "gcvid": _bf16((g ** C) * np.eye(128, dtype=np.float32)),
            "gpbf": _bf16(gpb),
            "gpt": gpt,
        })

    ncore = int(os.environ.get("K_NCORES", "8"))
    res = bass_utils.run_bass_kernel_spmd(
        nc, in_maps[:ncore], core_ids=list(range(ncore)),
        trace=bool(os.environ.get("K_TRACE")))
    global LAST_EXEC_NS, LAST_TRACE
    LAST_EXEC_NS = res.exec_time_ns
    LAST_TRACE = res.instructions_and_trace
    final = out.copy()
    for ci in range(len(res.results)):
        b = ci // 4
        final[b] += res.results[ci]["partial"].astype(np.float32)
    return final


# revision 52
# speedup vs baseline: 2.8711x; 1.0492x over previous
"""DeltaHebbianBlock Trainium2 kernel (v4, 254.5us vs 730.8us baseline).

Sharding: 8 cores = (B=2) x (H=4). Each core runs its head's delta-rule
chunked scan (C=128, degree-3 UT chain, rel_err ~1.1e-3 vs the 2e-2 gate)
and the partial output projection partial_bh = (alpha_h*o_bh) @ Wr_h^T.
Host gathers: out[b] = x[b] + sum_h partial (partial stored bf16).

Fully fused slot schedule per quarter (QT=1024, 8 chunks of C=128): slot n
emits P2-grams(q,n), P3-scan(q,n-3), P2-chain-tail one slot late (so the
scan's chain-critical PSUM drains lead the DVE/Act queues), P4 out-proj
(lag 8), and P1 v-proj/rk for q+1.  Key tricks:
- fp8e4m3 + DoubleRow matmuls for both DxD projections (K=256/instr,
  0.5 cyc/row); alpha folded into the oT drain scale, NOT the fp8 wrt
  (subnormal flush).
- wkcT stored negated and vcp/identity matmuls folded into the vnp PSUM
  group -> vnew accumulates fully in PSUM (no separate TSP).
- S decay folded into the sup matmul group via a gcv*I constant; S drain
  is a plain copy split jv-wise across Act/DVE (two interleaved chains).
- scan emits oT directly via transposed matmuls (no output transposes);
  (I+A0)(I+A0^2) chain with identity folds (no G0/Gh0 adds).
- PSUM: 8 banks exactly; one matmul group per bank at a time.
- GPSIMD/Pool cannot touch PSUM; it handles SBUF-only scalings.
"""
import os
import numpy as np
import ml_dtypes
from contextlib import ExitStack

import concourse.bass as bass
import concourse.mybir as mybir
import concourse.tile as tile
from concourse import bacc, bass_utils

B, T, D = 2, 8192, 1024
H, d, C = 4, 256, 128
NQ = 8                # quarter passes
QT = T // NQ          # 1024 tokens per pass
NCH = QT // C         # 8 chunks per pass

F32 = mybir.dt.float32
BF16 = mybir.dt.bfloat16
F8 = mybir.dt.float8e4
MULT = mybir.AluOpType.mult
ADD = mybir.AluOpType.add
ACT_COPY = None  # set in _build


def _build():
    nc = bacc.Bacc("TRN2", target_bir_lowering=False, debug=False,
                   num_devices=int(os.environ.get("K_NCORES", "8")))
    xT_d = nc.dram_tensor("xt", (D, T), F8, kind="ExternalInput")
    xh_d = nc.dram_tensor("xh", (T, d), BF16, kind="ExternalInput")
    wwt_d = nc.dram_tensor("wwt", (D, d), F8, kind="ExternalInput")
    wrt_d = nc.dram_tensor("wrt", (d, D), F8, kind="ExternalInput")
    alsc_d = nc.dram_tensor("alsc", (128, 1), F32, kind="ExternalInput")
    mb_d = nc.dram_tensor("mb", (C, C), F32, kind="ExternalInput")
    mc_d = nc.dram_tensor("mc", (C, C), F32, kind="ExternalInput")
    mit_d = nc.dram_tensor("mit", (C, C), F32, kind="ExternalInput")
    id_d = nc.dram_tensor("ident", (128, 128), BF16, kind="ExternalInput")
    gcvid_d = nc.dram_tensor("gcvid", (128, 128), BF16, kind="ExternalInput")
    gpb_d = nc.dram_tensor("gpbf", (128, QT), BF16, kind="ExternalInput")
    gpt_d = nc.dram_tensor("gpt", (128, 1), F32, kind="ExternalInput")
    part_d = nc.dram_tensor("partial", (T, D), BF16, kind="ExternalOutput")

    COPY = mybir.ActivationFunctionType.Copy
    SQRT = mybir.ActivationFunctionType.Sqrt
    GP = nc.gpsimd if os.environ.get("K_POOL", "1") == "1" else nc.vector

    with ExitStack() as ctx:
        tc = ctx.enter_context(tile.TileContext(nc))
        consts = ctx.enter_context(tc.tile_pool(name="consts", bufs=1))
        qx = ctx.enter_context(tc.tile_pool(name="qx", bufs=2))
        qa = ctx.enter_context(tc.tile_pool(name="qa", bufs=2))
        chp = ctx.enter_context(tc.tile_pool(name="chp", bufs=3))
        vnw = ctx.enter_context(tc.tile_pool(name="vnw", bufs=4))
        st_p = ctx.enter_context(tc.tile_pool(name="stp", bufs=2))
        scr = ctx.enter_context(tc.tile_pool(name="scr", bufs=2))
        ps = ctx.enter_context(tc.tile_pool(name="ps", bufs=1, space="PSUM"))

        # ---- constants / weights in SBUF ----
        wwt_s = consts.tile([128, 8, d], F8)
        nc.sync.dma_start(wwt_s[:], wwt_d.ap().rearrange("(kb p) j -> p kb j", p=128))
        wrt_s = consts.tile([128, 2, D], F8)
        nc.sync.dma_start(wrt_s[:], wrt_d.ap().rearrange("(kt p) n -> p kt n", p=128))
        mb2_s = consts.tile([128, 2, 128], F32)
        mc2_s = consts.tile([128, 2, 128], F32)
        mit2_s = consts.tile([128, 2, 128], F32)
        id2_s = consts.tile([128, 2, 128], BF16)
        for ch in range(2):
            nc.sync.dma_start(mb2_s[:, ch, :], mb_d.ap())
            nc.sync.dma_start(mc2_s[:, ch, :], mc_d.ap())
            nc.sync.dma_start(mit2_s[:, ch, :], mit_d.ap())
            nc.sync.dma_start(id2_s[:, ch, :], id_d.ap())
        id_s = consts.tile([128, 128], BF16)
        nc.sync.dma_start(id_s[:], id_d.ap())
        gcvid_s = consts.tile([128, 128], BF16)
        nc.sync.dma_start(gcvid_s[:], gcvid_d.ap())
        gpb_s = consts.tile([128, QT], BF16)
        nc.sync.dma_start(gpb_s[:], gpb_d.ap())
        gpt_s = consts.tile([128, 1], F32)
        nc.sync.dma_start(gpt_s[:], gpt_d.ap())
        alsc_s = consts.tile([128, 1], F32)
        nc.sync.dma_start(alsc_s[:], alsc_d.ap())
        ones_s = consts.tile([128, 1], BF16)
        nc.gpsimd.memset(ones_s[:], 1.0)

        S_bf = consts.tile([128, 2, d], BF16)
        nc.gpsimd.memset(S_bf[:], 0.0)

        QS = {}  # per-quarter tile sets

        def qtiles(qq):
            if qq in QS:
                return QS[qq]
            t = {}
            t["xT"] = qx.tile([128, 8, QT], F8, tag="xT", name="xT")
            t["xh"] = qa.tile([128, 8, d], BF16, tag="xh", name="xh")
            t["v_nat"] = qa.tile([128, 8, d], BF16, tag="v_nat", name="v_nat")
            t["nrm2"] = scr.tile([128, 8], F32, tag="nrm2", name="nrm2", bufs=3)
            t["nrm"] = scr.tile([128, 8], F32, tag="nrm", name="nrm", bufs=3)
            t["inv"] = scr.tile([128, 8], F32, tag="inv", name="inv", bufs=3)
            t["rk"] = qa.tile([128, 8, d], BF16, tag="rk", name="rk")
            t["wk"] = qa.tile([128, 8, d], BF16, tag="wk", name="wk")
            t["wkgN"] = qa.tile([128, 8, d], BF16, tag="wkgN", name="wkgN")
            t["rkT"] = qa.tile([128, 2, QT + 1], BF16, tag="rkT", name="rkT")
            t["rkgT"] = qa.tile([128, 2, QT], BF16, tag="rkgT", name="rkgT")
            t["wkcT"] = qa.tile([128, 2, QT], BF16, tag="wkcT", name="wkcT")
            t["inT"] = qa.tile([128, NCH, C], BF16, tag="inT", name="inT")
            t["AT"] = qa.tile([128, NCH, C], BF16, tag="AT", name="AT")
            t["oT"] = qa.tile([128, 2, QT], F8, tag="oT", name="oT")
            QS[qq] = t
            return t

        def loads(qq):
            t = qtiles(qq)
            qt0 = qq * QT
            nc.sync.dma_start(
                t["xT"][:], xT_d.ap()[:, qt0:qt0 + QT].rearrange(
                    "(kb p) t -> p kb t", p=128))
            nc.sync.dma_start(
                t["xh"][:], xh_d.ap()[qt0:qt0 + QT, :].rearrange(
                    "(tt p) j -> p tt j", p=128))


        # ---------- P1: v-proj + rk for token-tile n ----------
        def p1_slice(qq, n):
            t = qtiles(qq)
            vps = ps.tile([128, d], F32, tag="bigp", bufs=2, name="vps")
            for kp in range(4):
                nc.tensor.matmul(vps[:], t["xT"][:, 2 * kp:2 * kp + 2, n * 128:(n + 1) * 128],
                                 wwt_s[:, 2 * kp:2 * kp + 2, :], start=(kp == 0), stop=(kp == 3),
                                 perf_mode=mybir.MatmulPerfMode.DoubleRow)
            nc.scalar.activation(t["v_nat"][:, n, :], vps[:], COPY)
            sq = scr.tile([128, d], F32, tag="sq", name="sq")
            nc.scalar.activation(sq[:], t["xh"][:, n, :],
                                 mybir.ActivationFunctionType.Square,
                                 accum_out=t["nrm2"][:, n:n + 1])
            nc.scalar.activation(t["nrm"][:, n:n + 1], t["nrm2"][:, n:n + 1], SQRT)
            nc.vector.reciprocal(t["inv"][:, n:n + 1], t["nrm"][:, n:n + 1])
            nc.gpsimd.tensor_scalar(t["rk"][:, n, :], t["xh"][:, n, :],
                                    t["inv"][:, n:n + 1], None, MULT)
            tp = ps.tile([128, 2, 128], BF16, tag="cgtp", bufs=1, name="tp")
            for kt in range(2):
                nc.tensor.transpose(tp[:, kt, :],
                                    t["rk"][:, n, kt * 128:(kt + 1) * 128], id_s[:])
            nc.vector.tensor_copy(
                t["rkT"][:, :, 1 + n * 128:1 + (n + 1) * 128], tp[:])

        # ---------- dprep: shift/scale prep for quarter qq ----------
        def dprep(qq):
            t = qtiles(qq)
            if qq == 0:
                nc.gpsimd.memset(t["rkT"][:, :, 0:1], 0.0)
                nc.gpsimd.memset(t["wk"][0:1, 0:1, :], 0.0)
            else:
                tprev = QS[qq - 1]
                nc.vector.tensor_copy(t["rkT"][:, :, 0:1],
                                      tprev["rkT"][:, :, QT:QT + 1])
                nc.sync.dma_start(t["wk"][0:1, 0:1, :],
                                  tprev["rk"][127:128, 7:8, :])
            nc.sync.dma_start(t["wk"][1:128, :, :], t["rk"][0:127, :, :])
            nc.sync.dma_start(t["wk"][0:1, 1:8, :], t["rk"][127:128, 0:7, :])
            nc.gpsimd.tensor_scalar(t["wkgN"][:], t["wk"][:], gpt_s[:, 0:1],
                                    None, MULT)
            for kt in range(2):
                nc.gpsimd.tensor_mul(t["rkgT"][:, kt, :],
                                     t["rkT"][:, kt, 1:QT + 1], gpb_s[:])

        # ---------- P2: chain, pair-batched (called per slot) ----------
        # pair state carried between even/odd slots
        pair = {}

        def p2_slot(qq, n):
            t = qtiles(qq)
            half = n % 2
            w0 = n * C
            if half == 0:
                pair["g"] = ps.tile([128, 2, 2, 128], F32, tag="g", bufs=2,
                                    name="gpair")
            g = pair["g"]
            for kt in range(2):
                nc.tensor.matmul(g[:, half, 0, :], t["rkT"][:, kt, w0:w0 + 128],
                                 t["rkT"][:, kt, w0:w0 + 128],
                                 start=(kt == 0), stop=(kt == 1))
            for kt in range(2):
                nc.tensor.matmul(g[:, half, 1, :], t["rkT"][:, kt, w0:w0 + 128],
                                 t["rkT"][:, kt, w0 + 1:w0 + 129],
                                 start=(kt == 0), stop=(kt == 1))
            if half == 0:
                return
            # odd slot: drains + chain for the pair (chunks n-1, n)
            p0 = n - 1
            B0 = chp.tile([128, 2, 128], BF16, tag="B0", name="B0")
            nc.vector.tensor_mul(B0[:], g[:, :, 0, :], mb2_s[:])
            C0 = chp.tile([128, 2, 128], BF16, tag="C0", name="C0")
            nc.vector.tensor_mul(C0[:], g[:, :, 0, :], mc2_s[:])
            nc.vector.tensor_mul(t["inT"][:, p0:p0 + 2, :], g[:, :, 1, :],
                                 mit2_s[:])
            c1p = ps.tile([128, 2, 128], F32, tag="cgtp", bufs=1, name="c1p")
            for ch in range(2):
                nc.tensor.matmul(c1p[:, ch, :], B0[:, ch, :], C0[:, ch, :])
            C1 = chp.tile([128, 2, 128], BF16, tag="C1", name="C1")
            nc.scalar.activation(C1[:], c1p[:], COPY)
            # g1p = (I + B0)^T C1 = C1 + C0 C1 ; AT' = g1p + C0 = A^T - I
            g1p = ps.tile([128, 2, 128], F32, tag="cgtp", bufs=1, name="g1p")
            for ch in range(2):
                nc.tensor.matmul(g1p[:, ch, :], id_s[:], C1[:, ch, :],
                                 start=True, stop=False)
                nc.tensor.matmul(g1p[:, ch, :], B0[:, ch, :], C1[:, ch, :],
                                 start=False, stop=True)
            nc.vector.tensor_add(t["AT"][:, p0:p0 + 2, :], g1p[:], C0[:])
            wcp = ps.tile([128, 2, 2, 128], F32, tag="g", bufs=2, name="wcp")
            for ch in range(2):
                for jb in range(2):
                    nc.tensor.matmul(
                        wcp[:, ch, jb, :],
                        t["wk"][:, p0 + ch, jb * 128:(jb + 1) * 128],
                        t["AT"][:, p0 + ch, :])
            # negated store: wkcT = -(A wk)^T = -(wcp + wk^T)
            for ch in range(2):
                c0 = w0 - C + ch * 128
                nc.vector.scalar_tensor_tensor(
                    t["wkcT"][:, :, c0:c0 + 128], wcp[:, ch, :, :], -1.0,
                    t["rkT"][:, :, c0:c0 + 128], MULT, mybir.AluOpType.subtract)

        # ---------- P3: scan chunk (jv-split chains) ----------
        def p3_chunk(qq, n):
            t = qtiles(qq)
            w0 = n * C
            vnp = ps.tile([128, 2, 128], F32, tag="scan", bufs=3, name="vnp")
            nc.tensor.matmul(vnp[:, :, :], id_s[:], t["v_nat"][:, n, :],
                             start=True, stop=False)
            nc.tensor.matmul(vnp[:, :, :], t["AT"][:, n, :], t["v_nat"][:, n, :],
                             start=False, stop=False)
            for jv in range(2):
                for jb in range(2):
                    nc.tensor.matmul(vnp[:, jv, :], t["wkcT"][:, jb, w0:w0 + 128],
                                     S_bf[:, jb, jv * 128:(jv + 1) * 128],
                                     start=False, stop=(jv == 1 and jb == 1))
            vnew = vnw.tile([128, d], BF16, tag="vnew", name="vnew")
            nc.scalar.activation(vnew[:, 0:128], vnp[:, 0, :], COPY)
            nc.vector.tensor_copy(vnew[:, 128:256], vnp[:, 1, :])
            ot = ps.tile([128, 2, 128], F32, tag="scan", bufs=3, name="ot")
            for jv in range(2):
                for jb in range(2):
                    nc.tensor.matmul(ot[:, jv, :],
                                     S_bf[:, jb, jv * 128:(jv + 1) * 128],
                                     t["rkgT"][:, jb, w0:w0 + 128],
                                     start=(jb == 0), stop=False)
                nc.tensor.matmul(ot[:, jv, :],
                                 vnew[:, jv * 128:(jv + 1) * 128],
                                 t["inT"][:, n, :], start=False, stop=True)
            nc.scalar.activation(t["oT"][:, :, w0:w0 + 128], ot[:], COPY,
                                 scale=alsc_s[:, 0:1])
            sup = ps.tile([128, 2, d], F32, tag="scan", bufs=3, name="sup")
            for jb in range(2):
                nc.tensor.matmul(sup[:, jb, :], gcvid_s[:], S_bf[:, jb, :],
                                 start=True, stop=False)
                for jv in range(2):
                    nc.tensor.matmul(sup[:, jb, jv * 128:(jv + 1) * 128],
                                     t["wkgN"][:, n, jb * 128:(jb + 1) * 128],
                                     vnew[:, jv * 128:(jv + 1) * 128],
                                     start=False, stop=(jv == 1))
            # S <- sup (gcv*S folded into matmul group)
            nc.vector.tensor_copy(S_bf[:, :, 0:128], sup[:, :, 0:128])
            nc.scalar.activation(S_bf[:, :, 128:256], sup[:, :, 128:256], COPY)

        # ---------- P4: out-projection ----------
        def p4_chunk(qq, n, st):
            t = qtiles(qq)
            for nh in range(2):
                pps = ps.tile([128, 512], F32, tag="bigp", bufs=2, name="pps")
                nc.tensor.matmul(pps[:], t["oT"][:, :, n * 128:(n + 1) * 128],
                                 wrt_s[:, :, nh * 512:(nh + 1) * 512],
                                 start=True, stop=True,
                                 perf_mode=mybir.MatmulPerfMode.DoubleRow)
                if nh == 0:
                    nc.vector.tensor_copy(st[:, n % 2, 0:512], pps[:])
                else:
                    nc.scalar.activation(st[:, n % 2, 512:1024], pps[:], COPY)
            if n % 2 == 1:
                roff = qq * QT + (n - 1) * 128
                nc.sync.dma_start(
                    part_d.ap()[roff:roff + 256, :].rearrange(
                        "(c p) j -> p c j", p=128), st[:])

        # ---------------- schedule ----------------
        loads(0)
        loads(1)
        for n in range(NCH):
            p1_slice(0, n)
        dprep(0)
        LOADS_AHEAD = True
        # P4(q, m) runs at slot m+4 of quarter q (oT(q, m) drained at m+2);
        # chunks 6,7 spill to slots 0,1 of the next quarter.
        p4q = []   # pending (qq, chunk) in order
        st_box = [None]

        def p4_push(qq, m):
            p4q.append((qq, m))

        def p4_pop():
            if not p4q:
                return
            qq, m = p4q.pop(0)
            if m % 2 == 0:
                st_box[0] = st_p.tile([128, 2, QT], BF16, tag="st", name="st")
            p4_chunk(qq, m, st_box[0])

        for q in range(NQ):
            if 2 <= q + 1 < NQ:
                loads(q + 1)
            for n in range(10):
                if n < 8:
                    p2_slot(q, n)
                if 2 <= n:
                    p3_chunk(q, n - 2)
                    p4_push(q, n - 2)
                LAG = int(os.environ.get("K_P4LAG", "8"))
                while p4q and (p4q[0][0] < q or n - p4q[0][1] >= LAG):
                    p4_pop()
                    break
                if q < NQ - 1 and n < 8:
                    p1_slice(q + 1, n)
            if q < NQ - 1:
                dprep(q + 1)
            if q >= 1:
                QS.pop(q - 1, None)
        while p4q:
            p4_pop()
    nc.compile()
    return nc


_NC = None
LAST_EXEC_NS = None
LAST_TRACE = None


def _bf16(a):
    return np.ascontiguousarray(np.asarray(a).astype(ml_dtypes.bfloat16))


def _f8(a):
    return np.ascontiguousarray(np.asarray(a).astype(ml_dtypes.float8_e4m3))


def kernel(out, Ww, Wr, decay, log_alpha):
    global _NC
    out = np.asarray(out, dtype=np.float32)
    Ww = np.asarray(Ww, dtype=np.float32)
    Wr = np.asarray(Wr, dtype=np.float32)
    decay = np.asarray(decay, dtype=np.float32)
    log_alpha = np.asarray(log_alpha, dtype=np.float32)
    gamma = 1.0 / (1.0 + np.exp(-decay.astype(np.float64)))
    alpha = np.exp(log_alpha.astype(np.float64))

    if _NC is None:
        _NC = _build()
    nc = _NC

    pc = np.arange(C)
    xT_b = [_f8(out[b].T) for b in range(B)]
    in_maps = []
    for ci in range(8):
        b, h = ci // 4, ci % 4
        g = gamma[h]
        Ls = np.tril(g ** np.maximum(pc[:, None] - pc[None, :], 0), -1)
        mb = (-Ls).astype(np.float32)
        mit = np.triu(g ** np.maximum(pc[None, :] - pc[:, None], 0), 1).astype(np.float32)
        gp = (g ** (np.arange(QT) % C)).astype(np.float32)
        gpb = np.broadcast_to(gp[None, :], (128, QT))
        gpt = (g ** (C - 1 - np.arange(128)))[:, None].astype(np.float32)
        in_maps.append({
            "xt": xT_b[b],
            "xh": _bf16(out[b][:, h * d:(h + 1) * d]),
            "wwt": _f8(Ww[h * d:(h + 1) * d, :].T),
            "wrt": _f8(Wr[:, h * d:(h + 1) * d].T),
            "alsc": np.full((128, 1), alpha[h], np.float32),
            "mb": mb, "mc": np.ascontiguousarray(mb.T),
            "mit": mit,
            "ident": _bf16(np.eye(128, dtype=np.float32)),
            "gcvid": _bf16((g ** C) * np.eye(128, dtype=np.float32)),
            "gpbf": _bf16(gpb),
            "gpt": gpt,
        })

    ncore = int(os.environ.get("K_NCORES", "8"))
    res = bass_utils.run_bass_kernel_spmd(
        nc, in_maps[:ncore], core_ids=list(range(ncore)),
        trace=bool(os.environ.get("K_TRACE")))
    global LAST_EXEC_NS, LAST_TRACE
    LAST_EXEC_NS = res.exec_time_ns
    LAST_TRACE = res.instructions_and_trace
    final = out.copy()
    for ci in range(len(res.results)):
        b = ci // 4
        final[b] += res.results[ci]["partial"].astype(np.float32)
    return final


# revision 54
# speedup vs baseline: 2.8883x; 1.0060x over previous
"""DeltaHebbianBlock Trainium2 kernel (v4, 254.5us vs 730.8us baseline).

Sharding: 8 cores = (B=2) x (H=4). Each core runs its head's delta-rule
chunked scan (C=128, degree-3 UT chain, rel_err ~1.1e-3 vs the 2e-2 gate)
and the partial output projection partial_bh = (alpha_h*o_bh) @ Wr_h^T.
Host gathers: out[b] = x[b] + sum_h partial (partial stored bf16).

Fully fused slot schedule per quarter (QT=1024, 8 chunks of C=128): slot n
emits P2-grams(q,n), P3-scan(q,n-3), P2-chain-tail one slot late (so the
scan's chain-critical PSUM drains lead the DVE/Act queues), P4 out-proj
(lag 8), and P1 v-proj/rk for q+1.  Key tricks:
- fp8e4m3 + DoubleRow matmuls for both DxD projections (K=256/instr,
  0.5 cyc/row); alpha folded into the oT drain scale, NOT the fp8 wrt
  (subnormal flush).
- wkcT stored negated and vcp/identity matmuls folded into the vnp PSUM
  group -> vnew accumulates fully in PSUM (no separate TSP).
- S decay folded into the sup matmul group via a gcv*I constant; S drain
  is a plain copy split jv-wise across Act/DVE (two interleaved chains).
- scan emits oT directly via transposed matmuls (no output transposes);
  (I+A0)(I+A0^2) chain with identity folds (no G0/Gh0 adds).
- PSUM: 8 banks exactly; one matmul group per bank at a time.
- GPSIMD/Pool cannot touch PSUM; it handles SBUF-only scalings.
"""
import os
import numpy as np
import ml_dtypes
from contextlib import ExitStack

import concourse.bass as bass
import concourse.mybir as mybir
import concourse.tile as tile
from concourse import bacc, bass_utils

B, T, D = 2, 8192, 1024
H, d, C = 4, 256, 128
NQ = 8                # quarter passes
QT = T // NQ          # 1024 tokens per pass
NCH = QT // C         # 8 chunks per pass

F32 = mybir.dt.float32
BF16 = mybir.dt.bfloat16
F8 = mybir.dt.float8e4
MULT = mybir.AluOpType.mult
ADD = mybir.AluOpType.add
ACT_COPY = None  # set in _build


def _build():
    nc = bacc.Bacc("TRN2", target_bir_lowering=False, debug=False,
                   num_devices=int(os.environ.get("K_NCORES", "8")))
    xT_d = nc.dram_tensor("xt", (D, T), F8, kind="ExternalInput")
    xh_d = nc.dram_tensor("xh", (T, d), BF16, kind="ExternalInput")
    wwt_d = nc.dram_tensor("wwt", (D, d), F8, kind="ExternalInput")
    wrt_d = nc.dram_tensor("wrt", (d, D), F8, kind="ExternalInput")
    alsc_d = nc.dram_tensor("alsc", (128, 1), F32, kind="ExternalInput")
    mb_d = nc.dram_tensor("mb", (C, C), F32, kind="ExternalInput")
    mc_d = nc.dram_tensor("mc", (C, C), F32, kind="ExternalInput")
    mit_d = nc.dram_tensor("mit", (C, C), F32, kind="ExternalInput")
    id_d = nc.dram_tensor("ident", (128, 128), BF16, kind="ExternalInput")
    gcvid_d = nc.dram_tensor("gcvid", (128, 128), BF16, kind="ExternalInput")
    gpb_d = nc.dram_tensor("gpbf", (128, QT), BF16, kind="ExternalInput")
    gpt_d = nc.dram_tensor("gpt", (128, 1), F32, kind="ExternalInput")
    part_d = nc.dram_tensor("partial", (T, D), BF16, kind="ExternalOutput")

    COPY = mybir.ActivationFunctionType.Copy
    SQRT = mybir.ActivationFunctionType.Sqrt
    GP = nc.gpsimd if os.environ.get("K_POOL", "1") == "1" else nc.vector

    with ExitStack() as ctx:
        tc = ctx.enter_context(tile.TileContext(nc))
        consts = ctx.enter_context(tc.tile_pool(name="consts", bufs=1))
        qx = ctx.enter_context(tc.tile_pool(name="qx", bufs=2))
        qa = ctx.enter_context(tc.tile_pool(name="qa", bufs=2))
        chp = ctx.enter_context(tc.tile_pool(name="chp", bufs=3))
        vnw = ctx.enter_context(tc.tile_pool(name="vnw", bufs=4))
        st_p = ctx.enter_context(tc.tile_pool(name="stp", bufs=2))
        scr = ctx.enter_context(tc.tile_pool(name="scr", bufs=2))
        ps = ctx.enter_context(tc.tile_pool(name="ps", bufs=1, space="PSUM"))

        # ---- constants / weights in SBUF ----
        wwt_s = consts.tile([128, 8, d], F8)
        nc.sync.dma_start(wwt_s[:], wwt_d.ap().rearrange("(kb p) j -> p kb j", p=128))
        wrt_s = consts.tile([128, 2, D], F8)
        nc.sync.dma_start(wrt_s[:], wrt_d.ap().rearrange("(kt p) n -> p kt n", p=128))
        mb2_s = consts.tile([128, 2, 128], F32)
        mc2_s = consts.tile([128, 2, 128], F32)
        mit2_s = consts.tile([128, 2, 128], F32)
        id2_s = consts.tile([128, 2, 128], BF16)
        for ch in range(2):
            nc.sync.dma_start(mb2_s[:, ch, :], mb_d.ap())
            nc.sync.dma_start(mc2_s[:, ch, :], mc_d.ap())
            nc.sync.dma_start(mit2_s[:, ch, :], mit_d.ap())
            nc.sync.dma_start(id2_s[:, ch, :], id_d.ap())
        id_s = consts.tile([128, 128], BF16)
        nc.sync.dma_start(id_s[:], id_d.ap())
        gcvid_s = consts.tile([128, 128], BF16)
        nc.sync.dma_start(gcvid_s[:], gcvid_d.ap())
        gpb_s = consts.tile([128, QT], BF16)
        nc.sync.dma_start(gpb_s[:], gpb_d.ap())
        gpt_s = consts.tile([128, 1], F32)
        nc.sync.dma_start(gpt_s[:], gpt_d.ap())
        alsc_s = consts.tile([128, 1], F32)
        nc.sync.dma_start(alsc_s[:], alsc_d.ap())
        ones_s = consts.tile([128, 1], BF16)
        nc.gpsimd.memset(ones_s[:], 1.0)

        S_bf = consts.tile([128, 2, d], BF16)
        nc.gpsimd.memset(S_bf[:], 0.0)

        QS = {}  # per-quarter tile sets

        def qtiles(qq):
            if qq in QS:
                return QS[qq]
            t = {}
            t["xT"] = qx.tile([128, 8, QT], F8, tag="xT", name="xT")
            t["xh"] = qa.tile([128, 8, d], BF16, tag="xh", name="xh")
            t["v_nat"] = qa.tile([128, 8, d], BF16, tag="v_nat", name="v_nat")
            t["nrm2"] = scr.tile([128, 8], F32, tag="nrm2", name="nrm2", bufs=3)
            t["nrm"] = scr.tile([128, 8], F32, tag="nrm", name="nrm", bufs=3)
            t["inv"] = scr.tile([128, 8], F32, tag="inv", name="inv", bufs=3)
            t["rk"] = qa.tile([128, 8, d], BF16, tag="rk", name="rk")
            t["wk"] = qa.tile([128, 8, d], BF16, tag="wk", name="wk")
            t["wkgN"] = qa.tile([128, 8, d], BF16, tag="wkgN", name="wkgN")
            t["rkT"] = qa.tile([128, 2, QT + 1], BF16, tag="rkT", name="rkT")
            t["rkgT"] = qa.tile([128, 2, QT], BF16, tag="rkgT", name="rkgT")
            t["wkcT"] = qa.tile([128, 2, QT], BF16, tag="wkcT", name="wkcT")
            t["inT"] = qa.tile([128, NCH, C], BF16, tag="inT", name="inT")
            t["AT"] = qa.tile([128, NCH, C], BF16, tag="AT", name="AT")
            t["oT"] = qa.tile([128, 2, QT], F8, tag="oT", name="oT")
            QS[qq] = t
            return t

        def loads(qq):
            t = qtiles(qq)
            qt0 = qq * QT
            nc.sync.dma_start(
                t["xT"][:], xT_d.ap()[:, qt0:qt0 + QT].rearrange(
                    "(kb p) t -> p kb t", p=128))
            nc.sync.dma_start(
                t["xh"][:], xh_d.ap()[qt0:qt0 + QT, :].rearrange(
                    "(tt p) j -> p tt j", p=128))


        # ---------- P1: v-proj + rk for token-tile n ----------
        def p1_slice(qq, n):
            t = qtiles(qq)
            vps = ps.tile([128, d], F32, tag="bigp", bufs=2, name="vps")
            for kp in range(4):
                nc.tensor.matmul(vps[:], t["xT"][:, 2 * kp:2 * kp + 2, n * 128:(n + 1) * 128],
                                 wwt_s[:, 2 * kp:2 * kp + 2, :], start=(kp == 0), stop=(kp == 3),
                                 perf_mode=mybir.MatmulPerfMode.DoubleRow)
            nc.scalar.activation(t["v_nat"][:, n, :], vps[:], COPY)
            sq = scr.tile([128, d], F32, tag="sq", name="sq")
            nc.scalar.activation(sq[:], t["xh"][:, n, :],
                                 mybir.ActivationFunctionType.Square,
                                 accum_out=t["nrm2"][:, n:n + 1])
            nc.scalar.activation(t["nrm"][:, n:n + 1], t["nrm2"][:, n:n + 1], SQRT)
            nc.vector.reciprocal(t["inv"][:, n:n + 1], t["nrm"][:, n:n + 1])
            nc.gpsimd.tensor_scalar(t["rk"][:, n, :], t["xh"][:, n, :],
                                    t["inv"][:, n:n + 1], None, MULT)
            tp = ps.tile([128, 2, 128], BF16, tag="cgtp", bufs=1, name="tp")
            for kt in range(2):
                nc.tensor.transpose(tp[:, kt, :],
                                    t["rk"][:, n, kt * 128:(kt + 1) * 128], id_s[:])
            nc.vector.tensor_copy(
                t["rkT"][:, :, 1 + n * 128:1 + (n + 1) * 128], tp[:])

        # ---------- dprep: shift/scale prep for quarter qq ----------
        def dprep(qq):
            t = qtiles(qq)
            if qq == 0:
                nc.gpsimd.memset(t["rkT"][:, :, 0:1], 0.0)
                nc.gpsimd.memset(t["wk"][0:1, 0:1, :], 0.0)
            else:
                tprev = QS[qq - 1]
                nc.vector.tensor_copy(t["rkT"][:, :, 0:1],
                                      tprev["rkT"][:, :, QT:QT + 1])
                nc.sync.dma_start(t["wk"][0:1, 0:1, :],
                                  tprev["rk"][127:128, 7:8, :])
            nc.sync.dma_start(t["wk"][1:128, :, :], t["rk"][0:127, :, :])
            nc.sync.dma_start(t["wk"][0:1, 1:8, :], t["rk"][127:128, 0:7, :])
            nc.gpsimd.tensor_scalar(t["wkgN"][:], t["wk"][:], gpt_s[:, 0:1],
                                    None, MULT)
            for kt in range(2):
                nc.gpsimd.tensor_mul(t["rkgT"][:, kt, :],
                                     t["rkT"][:, kt, 1:QT + 1], gpb_s[:])

        # ---------- P2: chain, pair-batched (called per slot) ----------
        # pair state carried between even/odd slots
        pair = {}

        def p2_slot(qq, n):
            t = qtiles(qq)
            half = n % 2
            w0 = n * C
            if half == 0:
                pair["g"] = ps.tile([128, 2, 2, 128], F32, tag="g", bufs=2,
                                    name="gpair")
            g = pair["g"]
            for kt in range(2):
                nc.tensor.matmul(g[:, half, 0, :], t["rkT"][:, kt, w0:w0 + 128],
                                 t["rkT"][:, kt, w0:w0 + 128],
                                 start=(kt == 0), stop=(kt == 1))
            for kt in range(2):
                nc.tensor.matmul(g[:, half, 1, :], t["rkT"][:, kt, w0:w0 + 128],
                                 t["rkT"][:, kt, w0 + 1:w0 + 129],
                                 start=(kt == 0), stop=(kt == 1))
            if half == 0:
                return
            # odd slot: drains + chain for the pair (chunks n-1, n)
            p0 = n - 1
            B0 = chp.tile([128, 2, 128], BF16, tag="B0", name="B0")
            nc.vector.tensor_mul(B0[:], g[:, :, 0, :], mb2_s[:])
            C0 = chp.tile([128, 2, 128], BF16, tag="C0", name="C0")
            nc.vector.tensor_mul(C0[:], g[:, :, 0, :], mc2_s[:])
            nc.vector.tensor_mul(t["inT"][:, p0:p0 + 2, :], g[:, :, 1, :],
                                 mit2_s[:])
            c1p = ps.tile([128, 2, 128], F32, tag="cgtp", bufs=1, name="c1p")
            for ch in range(2):
                nc.tensor.matmul(c1p[:, ch, :], B0[:, ch, :], C0[:, ch, :])
            C1 = chp.tile([128, 2, 128], BF16, tag="C1", name="C1")
            nc.scalar.activation(C1[:], c1p[:], COPY)
            # g1p = (I + B0)^T C1 = C1 + C0 C1 ; AT' = g1p + C0 = A^T - I
            g1p = ps.tile([128, 2, 128], F32, tag="cgtp", bufs=1, name="g1p")
            for ch in range(2):
                nc.tensor.matmul(g1p[:, ch, :], id_s[:], C1[:, ch, :],
                                 start=True, stop=False)
                nc.tensor.matmul(g1p[:, ch, :], B0[:, ch, :], C1[:, ch, :],
                                 start=False, stop=True)
            nc.vector.tensor_add(t["AT"][:, p0:p0 + 2, :], g1p[:], C0[:])
            wcp = ps.tile([128, 2, 2, 128], F32, tag="g", bufs=2, name="wcp")
            for ch in range(2):
                for jb in range(2):
                    nc.tensor.matmul(
                        wcp[:, ch, jb, :],
                        t["wk"][:, p0 + ch, jb * 128:(jb + 1) * 128],
                        t["AT"][:, p0 + ch, :])
            # negated store: wkcT = -(A wk)^T = -(wcp + wk^T)
            for ch in range(2):
                c0 = w0 - C + ch * 128
                nc.vector.scalar_tensor_tensor(
                    t["wkcT"][:, :, c0:c0 + 128], wcp[:, ch, :, :], -1.0,
                    t["rkT"][:, :, c0:c0 + 128], MULT, mybir.AluOpType.subtract)

        # ---------- P3: scan chunk (jv-split chains) ----------
        def p3_chunk(qq, n):
            t = qtiles(qq)
            w0 = n * C
            vnp = ps.tile([128, 2, 128], F32, tag="scan", bufs=3, name="vnp")
            nc.tensor.matmul(vnp[:, :, :], id_s[:], t["v_nat"][:, n, :],
                             start=True, stop=False)
            nc.tensor.matmul(vnp[:, :, :], t["AT"][:, n, :], t["v_nat"][:, n, :],
                             start=False, stop=False)
            for jv in range(2):
                for jb in range(2):
                    nc.tensor.matmul(vnp[:, jv, :], t["wkcT"][:, jb, w0:w0 + 128],
                                     S_bf[:, jb, jv * 128:(jv + 1) * 128],
                                     start=False, stop=(jv == 1 and jb == 1))
            vnew = vnw.tile([128, d], BF16, tag="vnew", name="vnew")
            nc.scalar.activation(vnew[:, 0:128], vnp[:, 0, :], COPY)
            nc.vector.tensor_copy(vnew[:, 128:256], vnp[:, 1, :])
            ot = ps.tile([128, 2, 128], F32, tag="scan", bufs=3, name="ot")
            for jv in range(2):
                for jb in range(2):
                    nc.tensor.matmul(ot[:, jv, :],
                                     S_bf[:, jb, jv * 128:(jv + 1) * 128],
                                     t["rkgT"][:, jb, w0:w0 + 128],
                                     start=(jb == 0), stop=False)
                nc.tensor.matmul(ot[:, jv, :],
                                 vnew[:, jv * 128:(jv + 1) * 128],
                                 t["inT"][:, n, :], start=False, stop=True)
            nc.scalar.activation(t["oT"][:, :, w0:w0 + 128], ot[:], COPY,
                                 scale=alsc_s[:, 0:1])
            sup = ps.tile([128, 2, d], F32, tag="scan", bufs=3, name="sup")
            for jb in range(2):
                nc.tensor.matmul(sup[:, jb, :], gcvid_s[:], S_bf[:, jb, :],
                                 start=True, stop=False)
                for jv in range(2):
                    nc.tensor.matmul(sup[:, jb, jv * 128:(jv + 1) * 128],
                                     t["wkgN"][:, n, jb * 128:(jb + 1) * 128],
                                     vnew[:, jv * 128:(jv + 1) * 128],
                                     start=False, stop=(jv == 1))
            # S <- sup (gcv*S folded into matmul group)
            nc.vector.tensor_copy(S_bf[:, :, 0:128], sup[:, :, 0:128])
            nc.scalar.activation(S_bf[:, :, 128:256], sup[:, :, 128:256], COPY)

        # ---------- P4: out-projection ----------
        def p4_chunk(qq, n, st):
            t = qtiles(qq)
            for nh in range(2):
                pps = ps.tile([128, 512], F32, tag="bigp", bufs=2, name="pps")
                nc.tensor.matmul(pps[:], t["oT"][:, :, n * 128:(n + 1) * 128],
                                 wrt_s[:, :, nh * 512:(nh + 1) * 512],
                                 start=True, stop=True,
                                 perf_mode=mybir.MatmulPerfMode.DoubleRow)
                if nh == 0:
                    nc.vector.tensor_copy(st[:, n % 2, 0:512], pps[:])
                else:
                    nc.scalar.activation(st[:, n % 2, 512:1024], pps[:], COPY)
            if n % 2 == 1:
                roff = qq * QT + (n - 1) * 128
                nc.sync.dma_start(
                    part_d.ap()[roff:roff + 256, :].rearrange(
                        "(c p) j -> p c j", p=128), st[:])

        # ---------------- schedule ----------------
        loads(0)
        loads(1)
        for n in range(NCH):
            p1_slice(0, n)
        dprep(0)
        LOADS_AHEAD = True
        # P4(q, m) runs at slot m+4 of quarter q (oT(q, m) drained at m+2);
        # chunks 6,7 spill to slots 0,1 of the next quarter.
        p4q = []   # pending (qq, chunk) in order
        st_box = [None]

        def p4_push(qq, m):
            p4q.append((qq, m))

        def p4_pop():
            if not p4q:
                return
            qq, m = p4q.pop(0)
            if m % 2 == 0:
                st_box[0] = st_p.tile([128, 2, QT], BF16, tag="st", name="st")
            p4_chunk(qq, m, st_box[0])

        for q in range(NQ):
            if 2 <= q + 1 < NQ:
                loads(q + 1)
            for n in range(10):
                if n < 8:
                    p2_slot(q, n)
                if 2 <= n:
                    p3_chunk(q, n - 2)
                    p4_push(q, n - 2)
                LAG = int(os.environ.get("K_P4LAG", "8"))
                while p4q and (p4q[0][0] < q or n - p4q[0][1] >= LAG):
                    p4_pop()
                    break
                if q < NQ - 1 and n < 8:
                    p1_slice(q + 1, n)
            if q < NQ - 1:
                dprep(q + 1)
            if q >= 1:
                QS.pop(q - 1, None)
        while p4q:
            p4_pop()
    nc.compile()
    return nc


_NC = None
LAST_EXEC_NS = None
LAST_TRACE = None


def _bf16(a):
    return np.ascontiguousarray(np.asarray(a).astype(ml_dtypes.bfloat16))


def _f8(a):
    return np.ascontiguousarray(np.asarray(a).astype(ml_dtypes.float8_e4m3))


def kernel(out, Ww, Wr, decay, log_alpha):
    global _NC
    out = np.asarray(out, dtype=np.float32)
    Ww = np.asarray(Ww, dtype=np.float32)
    Wr = np.asarray(Wr, dtype=np.float32)
    decay = np.asarray(decay, dtype=np.float32)
    log_alpha = np.asarray(log_alpha, dtype=np.float32)
    gamma = 1.0 / (1.0 + np.exp(-decay.astype(np.float64)))
    alpha = np.exp(log_alpha.astype(np.float64))

    if _NC is None:
        _NC = _build()
    nc = _NC

    pc = np.arange(C)
    xT_b = [_f8(out[b].T) for b in range(B)]
    in_maps = []
    for ci in range(8):
        b, h = ci // 4, ci % 4
        g = gamma[h]
        Ls = np.tril(g ** np.maximum(pc[:, None] - pc[None, :], 0), -1)
        mb = (-Ls).astype(np.float32)
        mit = np.triu(g ** np.maximum(pc[None, :] - pc[:, None], 0), 1).astype(np.float32)
        gp = (g ** (np.arange(QT) % C)).astype(np.float32)
        gpb = np.broadcast_to(gp[None, :], (128, QT))
        gpt = (g ** (C - 1 - np.arange(128)))[:, None].astype(np.float32)
        in_maps.append({
            "xt": xT_b[b],
            "xh": _bf16(out[b][:, h * d:(h + 1) * d]),
            "wwt": _f8(Ww[h * d:(h + 1) * d, :].T),
            "wrt": _f8(Wr[:, h * d:(h + 1) * d].T),
            "alsc": np.full((128, 1), alpha[h], np.float32),
            "mb": mb, "mc": np.ascontiguousarray(mb.T),
            "mit": mit,
            "ident": _bf16(np.eye(128, dtype=np.float32)),
            "gcvid": _bf16((g ** C) * np.eye(128, dtype=np.float32)),
            "gpbf": _bf16(gpb),
            "gpt": gpt,
        })

    ncore = int(os.environ.get("K_NCORES", "8"))
    res = bass_utils.run_bass_kernel_spmd(
        nc, in_maps[:ncore], core_ids=list(range(ncore)),
        trace=bool(os.environ.get("K_TRACE")))
    global LAST_EXEC_NS, LAST_TRACE
    LAST_EXEC_NS = res.exec_time_ns
    LAST_TRACE = res.instructions_and_trace
    final = out.copy()
    for ci in range(len(res.results)):
        b = ci // 4
        final[b] += res.results[ci]["partial"].astype(np.float32)
    return final
